# revision 1
# baseline (speedup 1.0000x reference)
"""Trainium2 Bass kernel for nn_Net_19619410608498 (EdgeConv GNN, 23 layers).

Algorithm (per EdgeConv layer, using max-commutation):
    y = x @ wt ;  z = x @ wp + (bt + bp)
    out = segment_max(y[src], dst) - y + z

Distribution: node-parallel across 8 cores. Nodes are degree-sorted into
groups of 128, groups banded by degree so every core's block-position j has
the SAME padded degree D(j) (identical SPMD shapes + perfect balance). Each
core computes y/z for its own nodes, an AllGather builds the full y-table in
DRAM (the AG output IS the gather table), then dma_gather fetches each
node's padded neighbor rows and DVE reduce_max computes the segment max.

All index/permutation work happens on host at trace time (indices are
runtime inputs, but the Bass program is built inside kernel()).
"""
import numpy as np

import os

import concourse.bacc as bacc
import concourse.bass as bass
import concourse.mybir as mybir
import concourse.tile as tile
from concourse import masks as bass_masks
from concourse.bass_utils import run_bass_kernel_spmd

F32 = mybir.dt.float32
_ADDR_SPACE = os.environ.get("KERNEL_TABLE_SPACE", "Shared")
_NLAYERS = int(os.environ.get("KERNEL_NLAYERS", "23"))
I16 = mybir.dt.int16
NC = 8
N_ALL, N_PMT, N_VOX = 8600, 600, 8000
NBG = 9   # g-phase blocks per core (72 groups total, 68 real)
NBV = 8   # v-phase blocks per core (64 groups total, 63 real)


# ---------------------------------------------------------------- host prep

def _grouping(dst, n, nb_loc):
    """Degree-sort nodes into groups of 128; band groups by degree so core r,
    block j holds group band[j]*8 + r. Returns (new2old [NC*nb_loc*128],
    old2new [n], D [nb_loc] padded degree per block position, adj tokens)."""
    n_groups = NC * nb_loc
    deg = np.bincount(dst, minlength=n)
    order = np.argsort(deg, kind='stable')        # ascending degree, old ids
    slots = n_groups * 128
    new2old = np.full(slots, -1, dtype=np.int64)

    # group k (k < ceil(n/128)) = order[128k : 128k+128]; group Db
    real_groups = (n + 127) // 128
    gDb = np.ones(n_groups, dtype=np.int64)
    for k in range(real_groups):
        nodes = order[128 * k:128 * k + 128]
        gDb[k] = max(1, deg[nodes].max())
    # sort groups by Db desc; band j = ranks [8j, 8j+8); core r gets band[8j+r]
    grank = np.argsort(-gDb, kind='stable')
    D = np.zeros(nb_loc, dtype=np.int64)
    for j in range(nb_loc):
        band = grank[8 * j:8 * j + 8]
        D[j] = gDb[band].max()
        for r in range(8):
            k = band[r]
            base = NC if False else 0  # noqa
            nid0 = r * nb_loc * 128 + j * 128
            if k < real_groups:
                nodes = order[128 * k:128 * k + 128]
                new2old[nid0:nid0 + len(nodes)] = nodes
    old2new = np.full(n, -1, dtype=np.int64)
    valid = new2old >= 0
    old2new[new2old[valid]] = np.nonzero(valid)[0]
    return new2old, old2new, D


def _tables(src, dst, n, new2old, old2new, D, nb_loc):
    """Per-core int16 token arrays (concatenated per-block), packed 16-wise."""
    order = np.argsort(dst, kind='stable')
    s_sorted = src[order]
    deg = np.bincount(dst, minlength=n)
    starts = np.zeros(n + 1, dtype=np.int64)
    starts[1:] = np.cumsum(deg)
    idx_per_core = []
    for r in range(NC):
        toks = []
        for j in range(nb_loc):
            d = int(D[j])
            tok = np.zeros((d, 128), dtype=np.int16)
            for p in range(128):
                nid = r * nb_loc * 128 + j * 128 + p
                old = new2old[nid]
                if old >= 0:
                    di = int(deg[old])
                    nbrs = old2new[s_sorted[starts[old]:starts[old] + di]]
                    tok[:di, p] = nbrs
                    tok[di:, p] = nid
                # else leave 0 (dummy slot -> gathers row 0, output unused)
            toks.append(tok.reshape(-1))
        idx_per_core.append(np.concatenate(toks))
    return idx_per_core


def _pack16(v):
    """[T] int16 -> [128, T//16]: token t at partition t%16, col t//16,
    replicated across the 8 q7-core partition groups."""
    T = len(v)
    assert T % 16 == 0
    a = v.reshape(T // 16, 16).T.astype(np.int16)    # [16, T//16]
    return np.ascontiguousarray(np.tile(a, (8, 1)))


def _layer_descs():
    """23 layers: (wname, F_in, F_out, phase, mask_after, last)"""
    L = []
    for i in range(8):
        L.append(dict(F_in=54, F_out=54, phase='g'))
    for i in range(4):                      # a1..a4
        L.append(dict(F_in=54, F_out=54, phase='v'))
    L.append(dict(F_in=54, F_out=25, phase='v'))          # a5
    for i in range(9):                      # a6..a14
        L.append(dict(F_in=25, F_out=25, phase='v'))
    L.append(dict(F_in=25, F_out=1, phase='v'))           # a15
    L[10]['mask'] = 'u2'   # after a3
    L[13]['mask'] = 'u3'   # after a6
    L[16]['mask'] = 'u4'   # after a9
    L[22]['last'] = True
    return L


# ---------------------------------------------------------------- bass build

def _build(Dg, Dv, Tg, Tv, descs):
    nc = bacc.Bacc("TRN2", target_bir_lowering=False, debug=False, num_devices=NC)

    x0_in = nc.dram_tensor("x0", [128, NBG, 64], F32, kind="ExternalInput")
    w_in = nc.dram_tensor("w", [64, 46 * 64], F32, kind="ExternalInput")
    idxg_in = nc.dram_tensor("idxg", [128, Tg // 16], I16, kind="ExternalInput")
    idxv_in = nc.dram_tensor("idxv", [128, Tv // 16], I16, kind="ExternalInput")
    idxt_in = nc.dram_tensor("idxt", [128, 64], I16, kind="ExternalInput")
    mask_in = {k: nc.dram_tensor(k, [128, NBV, 64], F32, kind="ExternalInput")
               for k in ('u1', 'u2', 'u3', 'u4')}
    out_dram = nc.dram_tensor("out", [128, NBV], F32, kind="ExternalOutput")
    dbg_dram = (nc.dram_tensor("dbg", [128, NBG, 64], F32, kind="ExternalOutput")
                if _NLAYERS < 23 else None)

    with tile.TileContext(nc) as tc:
        with (
            tc.tile_pool(name="const", bufs=1) as cpool,
            tc.tile_pool(name="stage", bufs=2) as stage_pool,
            tc.tile_pool(name="outT", bufs=2) as outT_pool,
            tc.tile_pool(name="ys", bufs=2) as y_pool,
            tc.tile_pool(name="zs", bufs=2) as z_pool,
            tc.tile_pool(name="ms", bufs=2) as m_pool,
            tc.tile_pool(name="gath", bufs=3) as gath_pool,
            tc.tile_pool(name="fin", bufs=1) as fin_pool,
            tc.tile_pool(name="psum", bufs=1, space="PSUM") as psum_pool,
            tc.tile_pool(name="agin", bufs=2, space="DRAM") as agin_pool,
            tc.tile_pool(name="tables", bufs=2, space="DRAM") as table_pool,
        ):
            ident = cpool.tile([128, 128], F32)
            bass_masks.make_identity(nc, ident[:])
            w_sb = cpool.tile([64, 46 * 64], F32)
            nc.sync.dma_start(w_sb[:], w_in[:])
            idxg_sb = cpool.tile([128, Tg // 16], I16)
            nc.sync.dma_start(idxg_sb[:], idxg_in[:])
            idxv_sb = cpool.tile([128, Tv // 16], I16)
            nc.sync.dma_start(idxv_sb[:], idxv_in[:])
            idxt_sb = cpool.tile([128, 64], I16)
            nc.sync.dma_start(idxt_sb[:], idxt_in[:])
            mask_sb = {}
            for k, t in mask_in.items():
                mask_sb[k] = cpool.tile([128, NBV, 64], F32, name=f"mask_{k}")
                nc.sync.dma_start(mask_sb[k][:], t[:])

            stage = stage_pool.tile([128, NBG, 64], F32)
            nc.sync.dma_start(stage[:], x0_in[:])

            for l, L in enumerate(descs):
                Fi, Fo = L['F_in'], L['F_out']
                phase = L['phase']
                NB = NBG if phase == 'g' else NBV
                D = Dg if phase == 'g' else Dv
                idx_sb = idxg_sb if phase == 'g' else idxv_sb

                # --- transpose own stage -> feat-major outT
                outT = outT_pool.tile([64, NBG * 128], F32)
                for j in range(NB):
                    pt = psum_pool.tile([64, 128], F32, tag="pt", bufs=2)
                    nc.tensor.matmul(pt[:], stage[:, j, :], ident[:],
                                     is_transpose=True)
                    nc.vector.tensor_copy(outT[:, j * 128:(j + 1) * 128], pt[:])

                # --- y / z matmuls (own nodes)
                y_st = y_pool.tile([128, NBG, 64], F32)
                z_st = z_pool.tile([128, NBG, 64], F32)
                wt_ap = w_sb[:Fi, (2 * l) * 64:(2 * l) * 64 + 64]
                wp_ap = w_sb[:Fi + 1, (2 * l + 1) * 64:(2 * l + 1) * 64 + 64]
                for j in range(NB):
                    sl = slice(j * 128, (j + 1) * 128)
                    py = psum_pool.tile([128, 64], F32, tag="py", bufs=3)
                    nc.tensor.matmul(py[:], outT[:Fi, sl], wt_ap)
                    nc.vector.tensor_copy(y_st[:, j, :], py[:])
                    pz = psum_pool.tile([128, 64], F32, tag="pz", bufs=3)
                    nc.tensor.matmul(pz[:], outT[:Fi + 1, sl], wp_ap)
                    nc.vector.tensor_copy(z_st[:, j, :], pz[:])

                # --- AllGather y -> table
                ag_in = agin_pool.tile([NBG * 128, 64], F32)
                nc.sync.dma_start(
                    ag_in[:NB * 128].rearrange("(j p) f -> p j f", p=128),
                    y_st[:, :NB, :])
                table = table_pool.tile([NC * NB * 128, 64], F32,
                                        addr_space=_ADDR_SPACE, tag="table",
                                        name=f"table{l}")
                nc.gpsimd.collective_compute(
                    "AllGather", mybir.AluOpType.bypass,
                    replica_groups=[list(range(NC))],
                    ins=[ag_in[:NB * 128, :]], outs=[table[:]])

                # --- gather + blockwise segment max
                m_st = m_pool.tile([128, NBG, 64], F32)
                off = 0
                for j in range(NB):
                    d = int(D[j])
                    g_t = gath_pool.tile([128, d, 64], F32, tag="gath")
                    nc.gpsimd.dma_gather(
                        g_t[:], table[:], idx_sb[:, off:off + d * 8],
                        d * 128, d * 128, 64, single_packet=False)
                    off += d * 8
                    nc.vector.reduce_max(
                        m_st[:, j, :Fo],
                        g_t[:, :, :Fo].rearrange("p d f -> p f d"),
                        axis=mybir.AxisListType.X)

                # --- pointwise: out = m - y + z  (+mask) (+ones col)
                msl = m_st[:, :NB, :Fo]
                nc.vector.tensor_sub(msl, msl, y_st[:, :NB, :Fo])
                if L.get('last'):
                    fin = fin_pool.tile([128, NBV], F32)
                    nc.vector.tensor_add(m_st[:, :NB, 0:1], msl,
                                         z_st[:, :NB, :Fo])
                    nc.vector.tensor_scalar_max(fin[:], m_st[:, :NB, 0], 0.0)
                    nc.sync.dma_start(out_dram[:], fin[:])
                    break
                new_stage = stage_pool.tile([128, NBG, 64], F32, tag="stage")
                nsl = new_stage[:, :NB, :Fo]
                nc.vector.tensor_add(nsl, msl, z_st[:, :NB, :Fo])
                if L.get('mask'):
                    nc.vector.tensor_mul(nsl, nsl,
                                         mask_sb[L['mask']][:, :NB, :Fo])
                nc.vector.memset(new_stage[:, :NB, Fo:Fo + 1], 1.0)
                if Fo + 1 < 64:
                    nc.vector.memset(new_stage[:, :NB, Fo + 1:], 0.0)
                if NB < NBG:
                    nc.vector.memset(new_stage[:, NB:, :], 0.0)
                stage = new_stage

                if l == _NLAYERS - 1 and dbg_dram is not None:
                    nc.sync.dma_start(dbg_dram[:, :NB, :], stage[:, :NB, :])
                    break

                # --- transition after b8: reshard g-space out -> v-space
                if l == 7:
                    ag2 = agin_pool.tile([NBG * 128, 64], F32, tag="agin")
                    nc.sync.dma_start(
                        ag2[:].rearrange("(j p) f -> p j f", p=128),
                        stage[:, :, :])
                    ttable = table_pool.tile([NC * NBG * 128, 64], F32,
                                             addr_space=_ADDR_SPACE, tag="table",
                                             name="ttable")
                    nc.gpsimd.collective_compute(
                        "AllGather", mybir.AluOpType.bypass,
                        replica_groups=[list(range(NC))],
                        ins=[ag2[:, :]], outs=[ttable[:]])
                    g_t = gath_pool.tile([128, NBV, 64], F32, tag="gath")
                    nc.gpsimd.dma_gather(
                        g_t[:], ttable[:], idxt_sb[:, :NBV * 8],
                        NBV * 128, NBV * 128, 64, single_packet=False)
                    stage_v = stage_pool.tile([128, NBG, 64], F32, tag="stage")
                    nc.vector.tensor_mul(stage_v[:, :NBV, :], g_t[:],
                                         mask_sb['u1'][:])
                    stage = stage_v

    nc.compile()
    return nc




def _bench_pjrt(nc, in_maps, n_iter=10):
    """Repeat execution with device-resident inputs; report per-run wall times.

    Mirrors bass2jax.run_bass_via_pjrt's multi-core path but keeps inputs on
    device so repeated calls measure launch + execute (not input shipping)."""
    import time as _time

    import jax
    from jax.sharding import Mesh, PartitionSpec
    from jax.experimental.shard_map import shard_map

    import concourse.mybir as _mybir
    from concourse import bass2jax as b2j

    b2j.install_neuronx_cc_hook()
    partition_name = nc.partition_id_tensor.name if nc.partition_id_tensor else None
    in_names, out_names, out_avals, zero_outs = [], [], [], []
    for alloc in nc.m.functions[0].allocations:
        if not isinstance(alloc, _mybir.MemoryLocationSet):
            continue
        name = alloc.memorylocations[0].name
        if alloc.kind == "ExternalInput":
            if name != partition_name:
                in_names.append(name)
        elif alloc.kind == "ExternalOutput":
            shape = tuple(alloc.tensor_shape)
            dtype = _mybir.dt.np(alloc.dtype)
            out_names.append(name)
            out_avals.append(jax.core.ShapedArray(shape, dtype))
            zero_outs.append(np.zeros(shape, dtype))
    n_params = len(in_names)
    n_outs = len(out_avals)
    all_names = list(in_names) + out_names + ([partition_name] if partition_name else [])

    def _body(*args):
        operands = list(args)
        if partition_name is not None:
            operands.append(b2j.partition_id_tensor())
        return tuple(b2j._bass_exec_p.bind(
            *operands, out_avals=tuple(out_avals), in_names=tuple(all_names),
            out_names=tuple(out_names), lowering_input_output_aliases=(),
            sim_require_finite=True, sim_require_nnan=True, nc=nc))

    devices = jax.devices()[:NC]
    mesh = Mesh(np.asarray(devices), ("core",))
    sharded = jax.jit(
        shard_map(_body, mesh=mesh,
                  in_specs=(PartitionSpec("core"),) * (n_params + n_outs),
                  out_specs=(PartitionSpec("core"),) * n_outs,
                  check_rep=False),
        donate_argnums=tuple(range(n_params, n_params + n_outs)),
        keep_unused=True)
    sharding = jax.sharding.NamedSharding(mesh, PartitionSpec("core"))
    dev_in = [jax.device_put(
        np.concatenate([np.asarray(m[k]) for m in in_maps], axis=0), sharding)
        for k in in_names]
    times = []
    for i in range(n_iter):
        zeros = [jax.device_put(
            np.zeros((NC * z.shape[0], *z.shape[1:]), z.dtype), sharding)
            for z in zero_outs]
        for z in zeros:
            z.block_until_ready()
        t0 = _time.perf_counter()
        outs = sharded(*dev_in, *zeros)
        for o in outs:
            o.block_until_ready()
        times.append(_time.perf_counter() - t0)
    return times


# ---------------------------------------------------------------- kernel

def kernel(**inputs):
    f32 = np.float32
    inp = {k: np.asarray(v) for k, v in inputs.items()}

    x0 = np.concatenate([
        np.concatenate([inp['inputs'][0].astype(f32),
                        np.zeros((N_VOX, 51), f32)], axis=0),
        inp['koor'].astype(f32)], axis=1)                  # [8600, 54]
    um = {k: ((inp[k] > 0.5).astype(f32) * 2.0) for k in ('u1', 'u2', 'u3', 'u4')}

    g_n2o, g_o2n, Dg = _grouping(inp['dst'], N_ALL, NBG)
    v_n2o, v_o2n, Dv = _grouping(inp['vdst'], N_VOX, NBV)
    idx_g = _tables(inp['src'], inp['dst'], N_ALL, g_n2o, g_o2n, Dg, NBG)
    idx_v = _tables(inp['vsrc'], inp['vdst'], N_VOX, v_n2o, v_o2n, Dv, NBV)
    Tg, Tv = 128 * int(Dg.sum()), 128 * int(Dv.sum())

    # per-core staged x0 (node-major, ones col at 54)
    x0_st = []
    for r in range(NC):
        st = np.zeros((128, NBG, 64), f32)
        st[:, :, 54] = 1.0
        for j in range(NBG):
            nid0 = r * NBG * 128 + j * 128
            olds = g_n2o[nid0:nid0 + 128]
            sel = olds >= 0
            st[sel, j, :54] = x0[olds[sel]]
        x0_st.append(st)

    # transition gather tokens: v-new-id -> g-new-id (of global old id 600+v)
    idxt = []
    for r in range(NC):
        tok = np.zeros(NBV * 128, np.int16)
        for j in range(NBV):
            for_p = v_n2o[r * NBV * 128 + j * 128: r * NBV * 128 + j * 128 + 128]
            t = np.zeros(128, np.int64)
            sel = for_p >= 0
            t[sel] = g_o2n[N_PMT + for_p[sel]]
            tok[j * 128:(j + 1) * 128] = t.astype(np.int16)
        idxt.append(_pack16(tok))

    # per-core masks (node-major padded)
    def mk_mask(u, F, ones_col):
        per = []
        for r in range(NC):
            mt = np.zeros((128, NBV, 64), f32)
            if ones_col is not None:
                mt[:, :, ones_col] = 1.0
            for j in range(NBV):
                olds = v_n2o[r * NBV * 128 + j * 128: r * NBV * 128 + j * 128 + 128]
                sel = olds >= 0
                mt[sel, j, :F] = u[olds[sel], :F]
            per.append(mt)
        return per
    m_u1 = mk_mask(um['u1'], 54, 54)
    m_u2 = mk_mask(um['u2'], 54, None)
    m_u3 = mk_mask(um['u3'], 25, None)
    m_u4 = mk_mask(um['u4'], 25, None)

    # weights: [64, 46*64]; layer l: wt at col 2l*64, wp_aug at (2l+1)*64
    descs = _layer_descs()
    wts = ([(inp['bwt'][i], inp['bbt'][i], inp['bwp'][i], inp['bbp'][i]) for i in range(8)]
           + [(inp['awt'][i], inp['abt'][i], inp['awp'][i], inp['abp'][i]) for i in range(4)]
           + [(inp['a5wt'], inp['a5bt'], inp['a5wp'], inp['a5bp'])]
           + [(inp['cwt'][i], inp['cbt'][i], inp['cwp'][i], inp['cbp'][i]) for i in range(9)]
           + [(inp['fwt'], inp['fbt'], inp['fwp'], inp['fbp'])])
    W = np.zeros((64, 46 * 64), f32)
    for l, (wt, bt, wp, bp) in enumerate(wts):
        Fi, Fo = wt.shape
        W[:Fi, 2 * l * 64:2 * l * 64 + Fo] = wt
        W[:Fi, (2 * l + 1) * 64:(2 * l + 1) * 64 + Fo] = wp
        W[Fi, (2 * l + 1) * 64:(2 * l + 1) * 64 + Fo] = bt + bp

    nc = _build(Dg, Dv, Tg, Tv, descs)

    in_maps = []
    for r in range(NC):
        in_maps.append({
            "x0": x0_st[r],
            "w": W,
            "idxg": _pack16(idx_g[r]),
            "idxv": _pack16(idx_v[r]),
            "idxt": idxt[r],
            "u1": m_u1[r], "u2": m_u2[r], "u3": m_u3[r], "u4": m_u4[r],
        })
    res = run_bass_kernel_spmd(nc, in_maps, core_ids=list(range(NC)))
    kernel.last_results = res
    nbench = int(os.environ.get("KERNEL_BENCH", "0"))
    if nbench:
        times = _bench_pjrt(nc, in_maps, nbench)
        kernel.bench_times = times
        print("bench ms:", " ".join(f"{t*1e3:.2f}" for t in times))

    out = np.zeros(N_VOX, f32)
    for r in range(NC):
        o = res.results[r]["out"]                  # [128, NBV]
        for j in range(NBV):
            olds = v_n2o[r * NBV * 128 + j * 128: r * NBV * 128 + j * 128 + 128]
            sel = olds >= 0
            out[olds[sel]] = o[sel, j]
    return out


if __name__ == "__main__":
    d = np.load('/tmp/inputs.npz')
    inputs = {k: d[k] for k in d.files}
    expected = np.load('/tmp/expected.npy')
    got = kernel(**inputs)
    rel = np.linalg.norm(got - expected) / np.linalg.norm(expected)
    print("rel_l2:", rel)



# revision 17
# speedup vs baseline: 1.0237x; 1.0237x over previous
"""Trainium2 Bass kernel for nn_Net_19619410608498 (EdgeConv GNN, 23 layers).

Algorithm (per EdgeConv layer, using max-commutation):
    y = x @ wt ;  z = x @ wp + (bt + bp)
    out = segment_max(y[src], dst) - y + z

Distribution: node-parallel across 8 cores. Nodes are degree-sorted into
groups of 128, groups banded by degree so every core's block-position j has
the SAME padded degree D(j) (identical SPMD shapes + perfect balance). Each
core computes y/z for its own nodes, an AllGather builds the full y-table in
DRAM (the AG output IS the gather table), then dma_gather fetches each
node's padded neighbor rows and DVE reduce_max computes the segment max.

All index/permutation work happens on host at trace time (indices are
runtime inputs, but the Bass program is built inside kernel()).
"""
import numpy as np

import os

import concourse.bacc as bacc
import concourse.bass as bass
import concourse.mybir as mybir
import concourse.tile as tile
from concourse import masks as bass_masks
from concourse.bass_utils import run_bass_kernel_spmd

F32 = mybir.dt.float32
_ADDR_SPACE = os.environ.get("KERNEL_TABLE_SPACE", "Shared")
_NLAYERS = int(os.environ.get("KERNEL_NLAYERS", "23"))
I16 = mybir.dt.int16
NC = 8
N_ALL, N_PMT, N_VOX = 8600, 600, 8000
NBG = 9   # g-phase blocks per core (72 groups total, 68 real)
NBV = 8   # v-phase blocks per core (64 groups total, 63 real)


# ---------------------------------------------------------------- host prep

def _grouping(dst, n, nb_loc):
    """Degree-sort nodes into groups of 128; band groups by degree so core r,
    block j holds group band[j]*8 + r. Returns (new2old [NC*nb_loc*128],
    old2new [n], D [nb_loc] padded degree per block position, adj tokens)."""
    n_groups = NC * nb_loc
    deg = np.bincount(dst, minlength=n)
    order = np.argsort(deg, kind='stable')        # ascending degree, old ids
    slots = n_groups * 128
    new2old = np.full(slots, -1, dtype=np.int64)

    # group k (k < ceil(n/128)) = order[128k : 128k+128]; group Db
    real_groups = (n + 127) // 128
    gDb = np.ones(n_groups, dtype=np.int64)
    for k in range(real_groups):
        nodes = order[128 * k:128 * k + 128]
        gDb[k] = max(1, deg[nodes].max())
    # sort groups by Db desc; band j = ranks [8j, 8j+8); core r gets band[8j+r]
    grank = np.argsort(-gDb, kind='stable')
    D = np.zeros(nb_loc, dtype=np.int64)
    for j in range(nb_loc):
        band = grank[8 * j:8 * j + 8]
        D[j] = gDb[band].max()
        for r in range(8):
            k = band[r]
            base = NC if False else 0  # noqa
            nid0 = r * nb_loc * 128 + j * 128
            if k < real_groups:
                nodes = order[128 * k:128 * k + 128]
                new2old[nid0:nid0 + len(nodes)] = nodes
    old2new = np.full(n, -1, dtype=np.int64)
    valid = new2old >= 0
    old2new[new2old[valid]] = np.nonzero(valid)[0]
    return new2old, old2new, D


def _tables(src, dst, n, new2old, old2new, D, nb_loc):
    """Per-core int16 token arrays (concatenated per-block), packed 16-wise."""
    order = np.argsort(dst, kind='stable')
    s_sorted = src[order]
    deg = np.bincount(dst, minlength=n)
    starts = np.zeros(n + 1, dtype=np.int64)
    starts[1:] = np.cumsum(deg)
    idx_per_core = []
    for r in range(NC):
        toks = []
        for j in range(nb_loc):
            d = int(D[j])
            tok = np.zeros((d, 128), dtype=np.int16)
            for p in range(128):
                nid = r * nb_loc * 128 + j * 128 + p
                old = new2old[nid]
                if old >= 0:
                    di = int(deg[old])
                    nbrs = old2new[s_sorted[starts[old]:starts[old] + di]]
                    tok[:di, p] = nbrs
                    tok[di:, p] = nid
                # else leave 0 (dummy slot -> gathers row 0, output unused)
            toks.append(tok.reshape(-1))
        idx_per_core.append(np.concatenate(toks))
    return idx_per_core


def _pack16(v):
    """[T] int16 -> [128, T//16]: token t at partition t%16, col t//16,
    replicated across the 8 q7-core partition groups."""
    T = len(v)
    assert T % 16 == 0
    a = v.reshape(T // 16, 16).T.astype(np.int16)    # [16, T//16]
    return np.ascontiguousarray(np.tile(a, (8, 1)))


def _layer_descs():
    """23 layers: (wname, F_in, F_out, phase, mask_after, last)"""
    L = []
    for i in range(8):
        L.append(dict(F_in=54, F_out=54, phase='g'))
    for i in range(4):                      # a1..a4
        L.append(dict(F_in=54, F_out=54, phase='v'))
    L.append(dict(F_in=54, F_out=25, phase='v'))          # a5
    for i in range(9):                      # a6..a14
        L.append(dict(F_in=25, F_out=25, phase='v'))
    L.append(dict(F_in=25, F_out=1, phase='v'))           # a15
    L[10]['mask'] = 'u2'   # after a3
    L[13]['mask'] = 'u3'   # after a6
    L[16]['mask'] = 'u4'   # after a9
    L[22]['last'] = True
    return L


# ---------------------------------------------------------------- bass build

def _build(Dg, Dv, Tg, Tv, descs):
    nc = bacc.Bacc("TRN2", target_bir_lowering=False, debug=False, num_devices=NC)

    x0_in = nc.dram_tensor("x0", [128, NBG, 64], F32, kind="ExternalInput")
    w_in = nc.dram_tensor("w", [64, 46 * 64], F32, kind="ExternalInput")
    idxg_in = nc.dram_tensor("idxg", [128, Tg // 16], I16, kind="ExternalInput")
    idxv_in = nc.dram_tensor("idxv", [128, Tv // 16], I16, kind="ExternalInput")
    idxt_in = nc.dram_tensor("idxt", [128, 64], I16, kind="ExternalInput")
    mask_in = {k: nc.dram_tensor(k, [128, NBV, 64], F32, kind="ExternalInput")
               for k in ('u1', 'u2', 'u3', 'u4')}
    out_dram = nc.dram_tensor("out", [128, NBV], F32, kind="ExternalOutput")
    dbg_dram = (nc.dram_tensor("dbg", [128, NBG, 64], F32, kind="ExternalOutput")
                if _NLAYERS < 23 else None)

    with tile.TileContext(nc) as tc:
        with (
            tc.tile_pool(name="const", bufs=1) as cpool,
            tc.tile_pool(name="stage", bufs=2) as stage_pool,
            tc.tile_pool(name="outT", bufs=2) as outT_pool,
            tc.tile_pool(name="ys", bufs=2) as y_pool,
            tc.tile_pool(name="zs", bufs=2) as z_pool,
            tc.tile_pool(name="ms", bufs=2) as m_pool,
            tc.tile_pool(name="gath", bufs=3) as gath_pool,
            tc.tile_pool(name="fin", bufs=1) as fin_pool,
            tc.tile_pool(name="psum", bufs=1, space="PSUM") as psum_pool,
            tc.tile_pool(name="agin", bufs=2, space="DRAM") as agin_pool,
            tc.tile_pool(name="tables", bufs=2, space="DRAM") as table_pool,
        ):
            ident = cpool.tile([128, 128], F32)
            bass_masks.make_identity(nc, ident[:])
            w_sb = cpool.tile([64, 46 * 64], F32)
            nc.sync.dma_start(w_sb[:], w_in[:])
            idxg_sb = cpool.tile([128, Tg // 16], I16)
            nc.sync.dma_start(idxg_sb[:], idxg_in[:])
            idxv_sb = cpool.tile([128, Tv // 16], I16)
            nc.sync.dma_start(idxv_sb[:], idxv_in[:])
            idxt_sb = cpool.tile([128, 64], I16)
            nc.sync.dma_start(idxt_sb[:], idxt_in[:])
            mask_sb = {}
            for k, t in mask_in.items():
                mask_sb[k] = cpool.tile([128, NBV, 64], F32, name=f"mask_{k}")
                nc.sync.dma_start(mask_sb[k][:], t[:])

            stage = stage_pool.tile([128, NBG, 64], F32)
            nc.sync.dma_start(stage[:], x0_in[:])

            for l, L in enumerate(descs):
                Fi, Fo = L['F_in'], L['F_out']
                phase = L['phase']
                NB = NBG if phase == 'g' else NBV
                D = Dg if phase == 'g' else Dv
                idx_sb = idxg_sb if phase == 'g' else idxv_sb

                # --- transpose own stage -> feat-major outT
                outT = outT_pool.tile([64, NBG * 128], F32)
                for j in range(NB):
                    pt = psum_pool.tile([64, 128], F32, tag="pt", bufs=2)
                    nc.tensor.matmul(pt[:], stage[:, j, :], ident[:],
                                     is_transpose=True)
                    nc.vector.tensor_copy(outT[:, j * 128:(j + 1) * 128], pt[:])

                # --- y / z matmuls (own nodes)
                y_st = y_pool.tile([128, NBG, 64], F32)
                z_st = z_pool.tile([128, NBG, 64], F32)
                wt_ap = w_sb[:Fi, (2 * l) * 64:(2 * l) * 64 + 64]
                wp_ap = w_sb[:Fi + 1, (2 * l + 1) * 64:(2 * l + 1) * 64 + 64]
                for j in range(NB):
                    sl = slice(j * 128, (j + 1) * 128)
                    py = psum_pool.tile([128, 64], F32, tag="py", bufs=3)
                    nc.tensor.matmul(py[:], outT[:Fi, sl], wt_ap)
                    nc.vector.tensor_copy(y_st[:, j, :], py[:])
                    pz = psum_pool.tile([128, 64], F32, tag="pz", bufs=3)
                    nc.tensor.matmul(pz[:], outT[:Fi + 1, sl], wp_ap)
                    nc.vector.tensor_copy(z_st[:, j, :], pz[:])

                # --- AllGather y -> table
                ag_in = agin_pool.tile([NBG * 128, 64], F32)
                nc.sync.dma_start(
                    ag_in[:NB * 128].rearrange("(j p) f -> p j f", p=128),
                    y_st[:, :NB, :])
                table = table_pool.tile([NC * NB * 128, 64], F32,
                                        addr_space=_ADDR_SPACE, tag="table",
                                        name=f"table{l}")
                nc.gpsimd.collective_compute(
                    "AllGather", mybir.AluOpType.bypass,
                    replica_groups=[list(range(NC))],
                    ins=[ag_in[:NB * 128, :]], outs=[table[:]])

                # --- gather + blockwise segment max
                m_st = m_pool.tile([128, NBG, 64], F32)
                off = 0
                for j in range(NB):
                    d = int(D[j])
                    g_t = gath_pool.tile([128, d, 64], F32, tag="gath")
                    nc.gpsimd.dma_gather(
                        g_t[:], table[:], idx_sb[:, off:off + d * 8],
                        d * 128, d * 128, 64, single_packet=False)
                    off += d * 8
                    nc.vector.reduce_max(
                        m_st[:, j, :Fo],
                        g_t[:, :, :Fo].rearrange("p d f -> p f d"),
                        axis=mybir.AxisListType.X)

                # --- pointwise: out = max(m, y_own) - y + z (+mask) (+ones)
                msl = m_st[:, :NB, :Fo]
                nc.vector.tensor_max(msl, msl, y_st[:, :NB, :Fo])
                nc.vector.tensor_sub(msl, msl, y_st[:, :NB, :Fo])
                if L.get('last'):
                    fin = fin_pool.tile([128, NBV], F32)
                    nc.vector.tensor_add(m_st[:, :NB, 0:1], msl,
                                         z_st[:, :NB, :Fo])
                    nc.vector.tensor_scalar_max(fin[:], m_st[:, :NB, 0], 0.0)
                    nc.sync.dma_start(out_dram[:], fin[:])
                    break
                new_stage = stage_pool.tile([128, NBG, 64], F32, tag="stage")
                nsl = new_stage[:, :NB, :Fo]
                nc.vector.tensor_add(nsl, msl, z_st[:, :NB, :Fo])
                if L.get('mask'):
                    nc.vector.tensor_mul(nsl, nsl,
                                         mask_sb[L['mask']][:, :NB, :Fo])
                nc.vector.memset(new_stage[:, :NB, Fo:Fo + 1], 1.0)
                if Fo + 1 < 64:
                    nc.vector.memset(new_stage[:, :NB, Fo + 1:], 0.0)
                if NB < NBG:
                    nc.vector.memset(new_stage[:, NB:, :], 0.0)
                stage = new_stage

                if l == _NLAYERS - 1 and dbg_dram is not None:
                    nc.sync.dma_start(dbg_dram[:, :NB, :], stage[:, :NB, :])
                    break

                # --- transition after b8: reshard g-space out -> v-space
                if l == 7:
                    ag2 = agin_pool.tile([NBG * 128, 64], F32, tag="agin")
                    nc.sync.dma_start(
                        ag2[:].rearrange("(j p) f -> p j f", p=128),
                        stage[:, :, :])
                    ttable = table_pool.tile([NC * NBG * 128, 64], F32,
                                             addr_space=_ADDR_SPACE, tag="table",
                                             name="ttable")
                    nc.gpsimd.collective_compute(
                        "AllGather", mybir.AluOpType.bypass,
                        replica_groups=[list(range(NC))],
                        ins=[ag2[:, :]], outs=[ttable[:]])
                    g_t = gath_pool.tile([128, NBV, 64], F32, tag="gath")
                    nc.gpsimd.dma_gather(
                        g_t[:], ttable[:], idxt_sb[:, :NBV * 8],
                        NBV * 128, NBV * 128, 64, single_packet=False)
                    stage_v = stage_pool.tile([128, NBG, 64], F32, tag="stage")
                    nc.vector.tensor_mul(stage_v[:, :NBV, :], g_t[:],
                                         mask_sb['u1'][:])
                    stage = stage_v

    nc.compile()
    return nc




def _bench_pjrt(nc, in_maps, n_iter=10):
    """Repeat execution with device-resident inputs; report per-run wall times.

    Mirrors bass2jax.run_bass_via_pjrt's multi-core path but keeps inputs on
    device so repeated calls measure launch + execute (not input shipping)."""
    import time as _time

    import jax
    from jax.sharding import Mesh, PartitionSpec
    from jax.experimental.shard_map import shard_map

    import concourse.mybir as _mybir
    from concourse import bass2jax as b2j

    b2j.install_neuronx_cc_hook()
    partition_name = nc.partition_id_tensor.name if nc.partition_id_tensor else None
    in_names, out_names, out_avals, zero_outs = [], [], [], []
    for alloc in nc.m.functions[0].allocations:
        if not isinstance(alloc, _mybir.MemoryLocationSet):
            continue
        name = alloc.memorylocations[0].name
        if alloc.kind == "ExternalInput":
            if name != partition_name:
                in_names.append(name)
        elif alloc.kind == "ExternalOutput":
            shape = tuple(alloc.tensor_shape)
            dtype = _mybir.dt.np(alloc.dtype)
            out_names.append(name)
            out_avals.append(jax.core.ShapedArray(shape, dtype))
            zero_outs.append(np.zeros(shape, dtype))
    n_params = len(in_names)
    n_outs = len(out_avals)
    all_names = list(in_names) + out_names + ([partition_name] if partition_name else [])

    def _body(*args):
        operands = list(args)
        if partition_name is not None:
            operands.append(b2j.partition_id_tensor())
        return tuple(b2j._bass_exec_p.bind(
            *operands, out_avals=tuple(out_avals), in_names=tuple(all_names),
            out_names=tuple(out_names), lowering_input_output_aliases=(),
            sim_require_finite=True, sim_require_nnan=True, nc=nc))

    devices = jax.devices()[:NC]
    mesh = Mesh(np.asarray(devices), ("core",))
    sharded = jax.jit(
        shard_map(_body, mesh=mesh,
                  in_specs=(PartitionSpec("core"),) * (n_params + n_outs),
                  out_specs=(PartitionSpec("core"),) * n_outs,
                  check_rep=False),
        donate_argnums=tuple(range(n_params, n_params + n_outs)),
        keep_unused=True)
    sharding = jax.sharding.NamedSharding(mesh, PartitionSpec("core"))
    dev_in = [jax.device_put(
        np.concatenate([np.asarray(m[k]) for m in in_maps], axis=0), sharding)
        for k in in_names]
    times = []
    for i in range(n_iter):
        zeros = [jax.device_put(
            np.zeros((NC * z.shape[0], *z.shape[1:]), z.dtype), sharding)
            for z in zero_outs]
        for z in zeros:
            z.block_until_ready()
        t0 = _time.perf_counter()
        outs = sharded(*dev_in, *zeros)
        for o in outs:
            o.block_until_ready()
        times.append(_time.perf_counter() - t0)
    return times


# ---------------------------------------------------------------- kernel

def kernel(**inputs):
    f32 = np.float32
    inp = {k: np.asarray(v) for k, v in inputs.items()}

    x0 = np.concatenate([
        np.concatenate([inp['inputs'][0].astype(f32),
                        np.zeros((N_VOX, 51), f32)], axis=0),
        inp['koor'].astype(f32)], axis=1)                  # [8600, 54]
    um = {k: ((inp[k] > 0.5).astype(f32) * 2.0) for k in ('u1', 'u2', 'u3', 'u4')}

    # Self-edges only contribute y_own to the max; padding slots gather the
    # node's own row and an explicit max(m, y) covers full-degree blocks, so
    # drop them from the token tables entirely.
    gm = inp['src'] != inp['dst']
    vm = inp['vsrc'] != inp['vdst']
    g_src, g_dst = inp['src'][gm], inp['dst'][gm]
    v_src, v_dst = inp['vsrc'][vm], inp['vdst'][vm]
    g_n2o, g_o2n, Dg = _grouping(g_dst, N_ALL, NBG)
    v_n2o, v_o2n, Dv = _grouping(v_dst, N_VOX, NBV)
    idx_g = _tables(g_src, g_dst, N_ALL, g_n2o, g_o2n, Dg, NBG)
    idx_v = _tables(v_src, v_dst, N_VOX, v_n2o, v_o2n, Dv, NBV)
    Tg, Tv = 128 * int(Dg.sum()), 128 * int(Dv.sum())

    # per-core staged x0 (node-major, ones col at 54)
    x0_st = []
    for r in range(NC):
        st = np.zeros((128, NBG, 64), f32)
        st[:, :, 54] = 1.0
        for j in range(NBG):
            nid0 = r * NBG * 128 + j * 128
            olds = g_n2o[nid0:nid0 + 128]
            sel = olds >= 0
            st[sel, j, :54] = x0[olds[sel]]
        x0_st.append(st)

    # transition gather tokens: v-new-id -> g-new-id (of global old id 600+v)
    idxt = []
    for r in range(NC):
        tok = np.zeros(NBV * 128, np.int16)
        for j in range(NBV):
            for_p = v_n2o[r * NBV * 128 + j * 128: r * NBV * 128 + j * 128 + 128]
            t = np.zeros(128, np.int64)
            sel = for_p >= 0
            t[sel] = g_o2n[N_PMT + for_p[sel]]
            tok[j * 128:(j + 1) * 128] = t.astype(np.int16)
        idxt.append(_pack16(tok))

    # per-core masks (node-major padded)
    def mk_mask(u, F, ones_col):
        per = []
        for r in range(NC):
            mt = np.zeros((128, NBV, 64), f32)
            if ones_col is not None:
                mt[:, :, ones_col] = 1.0
            for j in range(NBV):
                olds = v_n2o[r * NBV * 128 + j * 128: r * NBV * 128 + j * 128 + 128]
                sel = olds >= 0
                mt[sel, j, :F] = u[olds[sel], :F]
            per.append(mt)
        return per
    m_u1 = mk_mask(um['u1'], 54, 54)
    m_u2 = mk_mask(um['u2'], 54, None)
    m_u3 = mk_mask(um['u3'], 25, None)
    m_u4 = mk_mask(um['u4'], 25, None)

    # weights: [64, 46*64]; layer l: wt at col 2l*64, wp_aug at (2l+1)*64
    descs = _layer_descs()
    wts = ([(inp['bwt'][i], inp['bbt'][i], inp['bwp'][i], inp['bbp'][i]) for i in range(8)]
           + [(inp['awt'][i], inp['abt'][i], inp['awp'][i], inp['abp'][i]) for i in range(4)]
           + [(inp['a5wt'], inp['a5bt'], inp['a5wp'], inp['a5bp'])]
           + [(inp['cwt'][i], inp['cbt'][i], inp['cwp'][i], inp['cbp'][i]) for i in range(9)]
           + [(inp['fwt'], inp['fbt'], inp['fwp'], inp['fbp'])])
    W = np.zeros((64, 46 * 64), f32)
    for l, (wt, bt, wp, bp) in enumerate(wts):
        Fi, Fo = wt.shape
        W[:Fi, 2 * l * 64:2 * l * 64 + Fo] = wt
        W[:Fi, (2 * l + 1) * 64:(2 * l + 1) * 64 + Fo] = wp
        W[Fi, (2 * l + 1) * 64:(2 * l + 1) * 64 + Fo] = bt + bp

    nc = _build(Dg, Dv, Tg, Tv, descs)

    in_maps = []
    for r in range(NC):
        in_maps.append({
            "x0": x0_st[r],
            "w": W,
            "idxg": _pack16(idx_g[r]),
            "idxv": _pack16(idx_v[r]),
            "idxt": idxt[r],
            "u1": m_u1[r], "u2": m_u2[r], "u3": m_u3[r], "u4": m_u4[r],
        })
    res = run_bass_kernel_spmd(nc, in_maps, core_ids=list(range(NC)))
    kernel.last_results = res
    nbench = int(os.environ.get("KERNEL_BENCH", "0"))
    if nbench:
        times = _bench_pjrt(nc, in_maps, nbench)
        kernel.bench_times = times
        print("bench ms:", " ".join(f"{t*1e3:.2f}" for t in times))

    out = np.zeros(N_VOX, f32)
    for r in range(NC):
        o = res.results[r]["out"]                  # [128, NBV]
        for j in range(NBV):
            olds = v_n2o[r * NBV * 128 + j * 128: r * NBV * 128 + j * 128 + 128]
            sel = olds >= 0
            out[olds[sel]] = o[sel, j]
    return out


if __name__ == "__main__":
    d = np.load('/tmp/inputs.npz')
    inputs = {k: d[k] for k in d.files}
    expected = np.load('/tmp/expected.npy')
    got = kernel(**inputs)
    rel = np.linalg.norm(got - expected) / np.linalg.norm(expected)
    print("rel_l2:", rel)



# revision 18
# speedup vs baseline: 1.0327x; 1.0088x over previous
"""Trainium2 Bass kernel for nn_Net_19619410608498 (EdgeConv GNN, 23 layers).

Algorithm (per EdgeConv layer, using max-commutation):
    y = x @ wt ;  z = x @ wp + (bt + bp)
    out = segment_max(y[src], dst) - y + z

Distribution: node-parallel across 8 cores. Nodes are degree-sorted into
groups of 128, groups banded by degree so every core's block-position j has
the SAME padded degree D(j) (identical SPMD shapes + perfect balance). Each
core computes y/z for its own nodes, an AllGather builds the full y-table in
DRAM (the AG output IS the gather table), then dma_gather fetches each
node's padded neighbor rows and DVE reduce_max computes the segment max.

All index/permutation work happens on host at trace time (indices are
runtime inputs, but the Bass program is built inside kernel()).
"""
import numpy as np

import os

import concourse.bacc as bacc
import concourse.bass as bass
import concourse.mybir as mybir
import concourse.tile as tile
from concourse import masks as bass_masks
from concourse.bass_utils import run_bass_kernel_spmd

F32 = mybir.dt.float32
_ADDR_SPACE = os.environ.get("KERNEL_TABLE_SPACE", "Shared")
_NLAYERS = int(os.environ.get("KERNEL_NLAYERS", "23"))
I16 = mybir.dt.int16
NC = 8
N_ALL, N_PMT, N_VOX = 8600, 600, 8000
NBG = 9   # g-phase blocks per core (72 groups total, 68 real)
NBV = 8   # v-phase blocks per core (64 groups total, 63 real)


# ---------------------------------------------------------------- host prep

def _grouping(dst, n, nb_loc):
    """Degree-sort nodes into groups of 128; band groups by degree so core r,
    block j holds group band[j]*8 + r. Returns (new2old [NC*nb_loc*128],
    old2new [n], D [nb_loc] padded degree per block position, adj tokens)."""
    n_groups = NC * nb_loc
    deg = np.bincount(dst, minlength=n)
    order = np.argsort(deg, kind='stable')        # ascending degree, old ids
    slots = n_groups * 128
    new2old = np.full(slots, -1, dtype=np.int64)

    # group k (k < ceil(n/128)) = order[128k : 128k+128]; group Db
    real_groups = (n + 127) // 128
    gDb = np.ones(n_groups, dtype=np.int64)
    for k in range(real_groups):
        nodes = order[128 * k:128 * k + 128]
        gDb[k] = max(1, deg[nodes].max())
    # sort groups by Db desc; band j = ranks [8j, 8j+8); core r gets band[8j+r]
    grank = np.argsort(-gDb, kind='stable')
    D = np.zeros(nb_loc, dtype=np.int64)
    for j in range(nb_loc):
        band = grank[8 * j:8 * j + 8]
        D[j] = gDb[band].max()
        for r in range(8):
            k = band[r]
            base = NC if False else 0  # noqa
            nid0 = r * nb_loc * 128 + j * 128
            if k < real_groups:
                nodes = order[128 * k:128 * k + 128]
                new2old[nid0:nid0 + len(nodes)] = nodes
    old2new = np.full(n, -1, dtype=np.int64)
    valid = new2old >= 0
    old2new[new2old[valid]] = np.nonzero(valid)[0]
    return new2old, old2new, D


def _tables(src, dst, n, new2old, old2new, D, nb_loc):
    """Per-core int16 token arrays (concatenated per-block), packed 16-wise."""
    order = np.argsort(dst, kind='stable')
    s_sorted = src[order]
    deg = np.bincount(dst, minlength=n)
    starts = np.zeros(n + 1, dtype=np.int64)
    starts[1:] = np.cumsum(deg)
    idx_per_core = []
    for r in range(NC):
        toks = []
        for j in range(nb_loc):
            d = int(D[j])
            tok = np.zeros((d, 128), dtype=np.int16)
            for p in range(128):
                nid = r * nb_loc * 128 + j * 128 + p
                old = new2old[nid]
                if old >= 0:
                    di = int(deg[old])
                    nbrs = old2new[s_sorted[starts[old]:starts[old] + di]]
                    tok[:di, p] = nbrs
                    tok[di:, p] = nid
                # else leave 0 (dummy slot -> gathers row 0, output unused)
            toks.append(tok.reshape(-1))
        idx_per_core.append(np.concatenate(toks))
    return idx_per_core


def _pack16(v):
    """[T] int16 -> [128, T//16]: token t at partition t%16, col t//16,
    replicated across the 8 q7-core partition groups."""
    T = len(v)
    assert T % 16 == 0
    a = v.reshape(T // 16, 16).T.astype(np.int16)    # [16, T//16]
    return np.ascontiguousarray(np.tile(a, (8, 1)))


def _layer_descs():
    """23 layers: (wname, F_in, F_out, phase, mask_after, last)"""
    L = []
    for i in range(8):
        L.append(dict(F_in=54, F_out=54, phase='g'))
    for i in range(4):                      # a1..a4
        L.append(dict(F_in=54, F_out=54, phase='v'))
    L.append(dict(F_in=54, F_out=25, phase='v'))          # a5
    for i in range(9):                      # a6..a14
        L.append(dict(F_in=25, F_out=25, phase='v'))
    L.append(dict(F_in=25, F_out=1, phase='v'))           # a15
    L[10]['mask'] = 'u2'   # after a3
    L[13]['mask'] = 'u3'   # after a6
    L[16]['mask'] = 'u4'   # after a9
    L[22]['last'] = True
    return L


# ---------------------------------------------------------------- bass build

def _build(Dg, Dv, Tg, Tv, descs):
    nc = bacc.Bacc("TRN2", target_bir_lowering=False, debug=False, num_devices=NC)

    x0_in = nc.dram_tensor("x0", [128, NBG, 64], F32, kind="ExternalInput")
    w_in = nc.dram_tensor("w", [64, 46 * 64], F32, kind="ExternalInput")
    idxg_in = nc.dram_tensor("idxg", [128, Tg // 16], I16, kind="ExternalInput")
    idxv_in = nc.dram_tensor("idxv", [128, Tv // 16], I16, kind="ExternalInput")
    idxt_in = nc.dram_tensor("idxt", [128, 64], I16, kind="ExternalInput")
    mask_in = {k: nc.dram_tensor(k, [128, NBV, 64], F32, kind="ExternalInput")
               for k in ('u1', 'u2', 'u3', 'u4')}
    out_dram = nc.dram_tensor("out", [128, NBV], F32, kind="ExternalOutput")
    dbg_dram = (nc.dram_tensor("dbg", [128, NBG, 64], F32, kind="ExternalOutput")
                if _NLAYERS < 23 else None)

    with tile.TileContext(nc) as tc:
        with (
            tc.tile_pool(name="const", bufs=1) as cpool,
            tc.tile_pool(name="stage", bufs=2) as stage_pool,
            tc.tile_pool(name="outT", bufs=2) as outT_pool,
            tc.tile_pool(name="ys", bufs=2) as y_pool,
            tc.tile_pool(name="zs", bufs=2) as z_pool,
            tc.tile_pool(name="ms", bufs=2) as m_pool,
            tc.tile_pool(name="gath", bufs=3) as gath_pool,
            tc.tile_pool(name="fin", bufs=1) as fin_pool,
            tc.tile_pool(name="psum", bufs=1, space="PSUM") as psum_pool,
            tc.tile_pool(name="agin", bufs=2, space="DRAM") as agin_pool,
            tc.tile_pool(name="tables", bufs=2, space="DRAM") as table_pool,
        ):
            ident = cpool.tile([128, 128], F32)
            bass_masks.make_identity(nc, ident[:])
            w_sb = cpool.tile([64, 46 * 64], F32)
            nc.sync.dma_start(w_sb[:], w_in[:])
            idxg_sb = cpool.tile([128, Tg // 16], I16)
            nc.sync.dma_start(idxg_sb[:], idxg_in[:])
            idxv_sb = cpool.tile([128, Tv // 16], I16)
            nc.sync.dma_start(idxv_sb[:], idxv_in[:])
            idxt_sb = cpool.tile([128, 64], I16)
            nc.sync.dma_start(idxt_sb[:], idxt_in[:])
            mask_sb = {}
            for k, t in mask_in.items():
                mask_sb[k] = cpool.tile([128, NBV, 64], F32, name=f"mask_{k}")
                nc.sync.dma_start(mask_sb[k][:], t[:])

            stage = stage_pool.tile([128, NBG, 64], F32)
            nc.sync.dma_start(stage[:], x0_in[:])

            for l, L in enumerate(descs):
                Fi, Fo = L['F_in'], L['F_out']
                phase = L['phase']
                NB = NBG if phase == 'g' else NBV
                D = Dg if phase == 'g' else Dv
                idx_sb = idxg_sb if phase == 'g' else idxv_sb

                # --- transpose own stage -> feat-major outT
                outT = outT_pool.tile([64, NBG * 128], F32)
                for j in range(NB):
                    pt = psum_pool.tile([64, 128], F32, tag="pt", bufs=2)
                    nc.tensor.matmul(pt[:], stage[:, j, :], ident[:],
                                     is_transpose=True)
                    nc.vector.tensor_copy(outT[:, j * 128:(j + 1) * 128], pt[:])

                # --- y / z matmuls (own nodes)
                y_st = y_pool.tile([128, NBG, 64], F32)
                z_st = z_pool.tile([128, NBG, 64], F32)
                wt_ap = w_sb[:Fi, (2 * l) * 64:(2 * l) * 64 + 64]
                wp_ap = w_sb[:Fi + 1, (2 * l + 1) * 64:(2 * l + 1) * 64 + 64]
                for j in range(NB):
                    sl = slice(j * 128, (j + 1) * 128)
                    py = psum_pool.tile([128, 64], F32, tag="py", bufs=3)
                    nc.tensor.matmul(py[:], outT[:Fi, sl], wt_ap)
                    nc.vector.tensor_copy(y_st[:, j, :], py[:])
                    pz = psum_pool.tile([128, 64], F32, tag="pz", bufs=3)
                    nc.tensor.matmul(pz[:], outT[:Fi + 1, sl], wp_ap)
                    nc.vector.tensor_copy(z_st[:, j, :], pz[:])

                # --- AllGather y -> table
                ag_in = agin_pool.tile([NBG * 128, 64], F32)
                nc.sync.dma_start(
                    ag_in[:NB * 128].rearrange("(j p) f -> p j f", p=128),
                    y_st[:, :NB, :])
                table = table_pool.tile([NC * NB * 128, 64], F32,
                                        addr_space=_ADDR_SPACE, tag="table",
                                        name=f"table{l}")
                nc.gpsimd.collective_compute(
                    "AllGather", mybir.AluOpType.bypass,
                    replica_groups=[list(range(NC))],
                    ins=[ag_in[:NB * 128, :]], outs=[table[:]])

                # --- gather + blockwise segment max, pointwise per half so
                # half A's pointwise (and the next layer's transposes) overlap
                # half B's gathers/reduces on gpsimd.
                m_st = m_pool.tile([128, NBG, 64], F32)
                last = L.get('last')
                new_stage = (None if last else
                             stage_pool.tile([128, NBG, 64], F32, tag="stage"))

                def pointwise(jsl):
                    msl = m_st[:, jsl, :Fo]
                    nc.vector.tensor_max(msl, msl, y_st[:, jsl, :Fo])
                    nc.vector.tensor_sub(msl, msl, y_st[:, jsl, :Fo])
                    nsl = new_stage[:, jsl, :Fo]
                    nc.vector.tensor_add(nsl, msl, z_st[:, jsl, :Fo])
                    if L.get('mask'):
                        nc.vector.tensor_mul(
                            nsl, nsl, mask_sb[L['mask']][:, jsl, :Fo])
                    nc.vector.memset(new_stage[:, jsl, Fo:Fo + 1], 1.0)
                    if Fo + 1 < 64:
                        nc.vector.memset(new_stage[:, jsl, Fo + 1:], 0.0)

                half = NB // 2 + 1
                off = 0
                for j in range(NB):
                    d = int(D[j])
                    g_t = gath_pool.tile([128, d, 64], F32, tag="gath")
                    nc.gpsimd.dma_gather(
                        g_t[:], table[:], idx_sb[:, off:off + d * 8],
                        d * 128, d * 128, 64, single_packet=False)
                    off += d * 8
                    nc.vector.reduce_max(
                        m_st[:, j, :Fo],
                        g_t[:, :, :Fo].rearrange("p d f -> p f d"),
                        axis=mybir.AxisListType.X)
                    if not last and j == half - 1:
                        pointwise(slice(0, half))

                if last:
                    msl = m_st[:, :NB, :Fo]
                    nc.vector.tensor_max(msl, msl, y_st[:, :NB, :Fo])
                    nc.vector.tensor_sub(msl, msl, y_st[:, :NB, :Fo])
                    fin = fin_pool.tile([128, NBV], F32)
                    nc.vector.tensor_add(m_st[:, :NB, 0:1], msl,
                                         z_st[:, :NB, :Fo])
                    nc.vector.tensor_scalar_max(fin[:], m_st[:, :NB, 0], 0.0)
                    nc.sync.dma_start(out_dram[:], fin[:])
                    break
                pointwise(slice(half, NB))
                if NB < NBG:
                    nc.vector.memset(new_stage[:, NB:, :], 0.0)
                stage = new_stage

                if l == _NLAYERS - 1 and dbg_dram is not None:
                    nc.sync.dma_start(dbg_dram[:, :NB, :], stage[:, :NB, :])
                    break

                # --- transition after b8: reshard g-space out -> v-space
                if l == 7:
                    ag2 = agin_pool.tile([NBG * 128, 64], F32, tag="agin")
                    nc.sync.dma_start(
                        ag2[:].rearrange("(j p) f -> p j f", p=128),
                        stage[:, :, :])
                    ttable = table_pool.tile([NC * NBG * 128, 64], F32,
                                             addr_space=_ADDR_SPACE, tag="table",
                                             name="ttable")
                    nc.gpsimd.collective_compute(
                        "AllGather", mybir.AluOpType.bypass,
                        replica_groups=[list(range(NC))],
                        ins=[ag2[:, :]], outs=[ttable[:]])
                    g_t = gath_pool.tile([128, NBV, 64], F32, tag="gath")
                    nc.gpsimd.dma_gather(
                        g_t[:], ttable[:], idxt_sb[:, :NBV * 8],
                        NBV * 128, NBV * 128, 64, single_packet=False)
                    stage_v = stage_pool.tile([128, NBG, 64], F32, tag="stage")
                    nc.vector.tensor_mul(stage_v[:, :NBV, :], g_t[:],
                                         mask_sb['u1'][:])
                    stage = stage_v

    nc.compile()
    return nc




def _bench_pjrt(nc, in_maps, n_iter=10):
    """Repeat execution with device-resident inputs; report per-run wall times.

    Mirrors bass2jax.run_bass_via_pjrt's multi-core path but keeps inputs on
    device so repeated calls measure launch + execute (not input shipping)."""
    import time as _time

    import jax
    from jax.sharding import Mesh, PartitionSpec
    from jax.experimental.shard_map import shard_map

    import concourse.mybir as _mybir
    from concourse import bass2jax as b2j

    b2j.install_neuronx_cc_hook()
    partition_name = nc.partition_id_tensor.name if nc.partition_id_tensor else None
    in_names, out_names, out_avals, zero_outs = [], [], [], []
    for alloc in nc.m.functions[0].allocations:
        if not isinstance(alloc, _mybir.MemoryLocationSet):
            continue
        name = alloc.memorylocations[0].name
        if alloc.kind == "ExternalInput":
            if name != partition_name:
                in_names.append(name)
        elif alloc.kind == "ExternalOutput":
            shape = tuple(alloc.tensor_shape)
            dtype = _mybir.dt.np(alloc.dtype)
            out_names.append(name)
            out_avals.append(jax.core.ShapedArray(shape, dtype))
            zero_outs.append(np.zeros(shape, dtype))
    n_params = len(in_names)
    n_outs = len(out_avals)
    all_names = list(in_names) + out_names + ([partition_name] if partition_name else [])

    def _body(*args):
        operands = list(args)
        if partition_name is not None:
            operands.append(b2j.partition_id_tensor())
        return tuple(b2j._bass_exec_p.bind(
            *operands, out_avals=tuple(out_avals), in_names=tuple(all_names),
            out_names=tuple(out_names), lowering_input_output_aliases=(),
            sim_require_finite=True, sim_require_nnan=True, nc=nc))

    devices = jax.devices()[:NC]
    mesh = Mesh(np.asarray(devices), ("core",))
    sharded = jax.jit(
        shard_map(_body, mesh=mesh,
                  in_specs=(PartitionSpec("core"),) * (n_params + n_outs),
                  out_specs=(PartitionSpec("core"),) * n_outs,
                  check_rep=False),
        donate_argnums=tuple(range(n_params, n_params + n_outs)),
        keep_unused=True)
    sharding = jax.sharding.NamedSharding(mesh, PartitionSpec("core"))
    dev_in = [jax.device_put(
        np.concatenate([np.asarray(m[k]) for m in in_maps], axis=0), sharding)
        for k in in_names]
    times = []
    for i in range(n_iter):
        zeros = [jax.device_put(
            np.zeros((NC * z.shape[0], *z.shape[1:]), z.dtype), sharding)
            for z in zero_outs]
        for z in zeros:
            z.block_until_ready()
        t0 = _time.perf_counter()
        outs = sharded(*dev_in, *zeros)
        for o in outs:
            o.block_until_ready()
        times.append(_time.perf_counter() - t0)
    return times


# ---------------------------------------------------------------- kernel

def kernel(**inputs):
    f32 = np.float32
    inp = {k: np.asarray(v) for k, v in inputs.items()}

    x0 = np.concatenate([
        np.concatenate([inp['inputs'][0].astype(f32),
                        np.zeros((N_VOX, 51), f32)], axis=0),
        inp['koor'].astype(f32)], axis=1)                  # [8600, 54]
    um = {k: ((inp[k] > 0.5).astype(f32) * 2.0) for k in ('u1', 'u2', 'u3', 'u4')}

    # Self-edges only contribute y_own to the max; padding slots gather the
    # node's own row and an explicit max(m, y) covers full-degree blocks, so
    # drop them from the token tables entirely.
    gm = inp['src'] != inp['dst']
    vm = inp['vsrc'] != inp['vdst']
    g_src, g_dst = inp['src'][gm], inp['dst'][gm]
    v_src, v_dst = inp['vsrc'][vm], inp['vdst'][vm]
    g_n2o, g_o2n, Dg = _grouping(g_dst, N_ALL, NBG)
    v_n2o, v_o2n, Dv = _grouping(v_dst, N_VOX, NBV)
    idx_g = _tables(g_src, g_dst, N_ALL, g_n2o, g_o2n, Dg, NBG)
    idx_v = _tables(v_src, v_dst, N_VOX, v_n2o, v_o2n, Dv, NBV)
    Tg, Tv = 128 * int(Dg.sum()), 128 * int(Dv.sum())

    # per-core staged x0 (node-major, ones col at 54)
    x0_st = []
    for r in range(NC):
        st = np.zeros((128, NBG, 64), f32)
        st[:, :, 54] = 1.0
        for j in range(NBG):
            nid0 = r * NBG * 128 + j * 128
            olds = g_n2o[nid0:nid0 + 128]
            sel = olds >= 0
            st[sel, j, :54] = x0[olds[sel]]
        x0_st.append(st)

    # transition gather tokens: v-new-id -> g-new-id (of global old id 600+v)
    idxt = []
    for r in range(NC):
        tok = np.zeros(NBV * 128, np.int16)
        for j in range(NBV):
            for_p = v_n2o[r * NBV * 128 + j * 128: r * NBV * 128 + j * 128 + 128]
            t = np.zeros(128, np.int64)
            sel = for_p >= 0
            t[sel] = g_o2n[N_PMT + for_p[sel]]
            tok[j * 128:(j + 1) * 128] = t.astype(np.int16)
        idxt.append(_pack16(tok))

    # per-core masks (node-major padded)
    def mk_mask(u, F, ones_col):
        per = []
        for r in range(NC):
            mt = np.zeros((128, NBV, 64), f32)
            if ones_col is not None:
                mt[:, :, ones_col] = 1.0
            for j in range(NBV):
                olds = v_n2o[r * NBV * 128 + j * 128: r * NBV * 128 + j * 128 + 128]
                sel = olds >= 0
                mt[sel, j, :F] = u[olds[sel], :F]
            per.append(mt)
        return per
    m_u1 = mk_mask(um['u1'], 54, 54)
    m_u2 = mk_mask(um['u2'], 54, None)
    m_u3 = mk_mask(um['u3'], 25, None)
    m_u4 = mk_mask(um['u4'], 25, None)

    # weights: [64, 46*64]; layer l: wt at col 2l*64, wp_aug at (2l+1)*64
    descs = _layer_descs()
    wts = ([(inp['bwt'][i], inp['bbt'][i], inp['bwp'][i], inp['bbp'][i]) for i in range(8)]
           + [(inp['awt'][i], inp['abt'][i], inp['awp'][i], inp['abp'][i]) for i in range(4)]
           + [(inp['a5wt'], inp['a5bt'], inp['a5wp'], inp['a5bp'])]
           + [(inp['cwt'][i], inp['cbt'][i], inp['cwp'][i], inp['cbp'][i]) for i in range(9)]
           + [(inp['fwt'], inp['fbt'], inp['fwp'], inp['fbp'])])
    W = np.zeros((64, 46 * 64), f32)
    for l, (wt, bt, wp, bp) in enumerate(wts):
        Fi, Fo = wt.shape
        W[:Fi, 2 * l * 64:2 * l * 64 + Fo] = wt
        W[:Fi, (2 * l + 1) * 64:(2 * l + 1) * 64 + Fo] = wp
        W[Fi, (2 * l + 1) * 64:(2 * l + 1) * 64 + Fo] = bt + bp

    nc = _build(Dg, Dv, Tg, Tv, descs)

    in_maps = []
    for r in range(NC):
        in_maps.append({
            "x0": x0_st[r],
            "w": W,
            "idxg": _pack16(idx_g[r]),
            "idxv": _pack16(idx_v[r]),
            "idxt": idxt[r],
            "u1": m_u1[r], "u2": m_u2[r], "u3": m_u3[r], "u4": m_u4[r],
        })
    res = run_bass_kernel_spmd(nc, in_maps, core_ids=list(range(NC)))
    kernel.last_results = res
    nbench = int(os.environ.get("KERNEL_BENCH", "0"))
    if nbench:
        times = _bench_pjrt(nc, in_maps, nbench)
        kernel.bench_times = times
        print("bench ms:", " ".join(f"{t*1e3:.2f}" for t in times))

    out = np.zeros(N_VOX, f32)
    for r in range(NC):
        o = res.results[r]["out"]                  # [128, NBV]
        for j in range(NBV):
            olds = v_n2o[r * NBV * 128 + j * 128: r * NBV * 128 + j * 128 + 128]
            sel = olds >= 0
            out[olds[sel]] = o[sel, j]
    return out


if __name__ == "__main__":
    d = np.load('/tmp/inputs.npz')
    inputs = {k: d[k] for k in d.files}
    expected = np.load('/tmp/expected.npy')
    got = kernel(**inputs)
    rel = np.linalg.norm(got - expected) / np.linalg.norm(expected)
    print("rel_l2:", rel)



# revision 19
# speedup vs baseline: 1.0338x; 1.0010x over previous
"""Trainium2 Bass kernel for nn_Net_19619410608498 (EdgeConv GNN, 23 layers).

Algorithm (per EdgeConv layer, using max-commutation):
    y = x @ wt ;  z = x @ wp + (bt + bp)
    out = segment_max(y[src], dst) - y + z

Distribution: node-parallel across 8 cores. Nodes are degree-sorted into
groups of 128, groups banded by degree so every core's block-position j has
the SAME padded degree D(j) (identical SPMD shapes + perfect balance). Each
core computes y/z for its own nodes, an AllGather builds the full y-table in
DRAM (the AG output IS the gather table), then dma_gather fetches each
node's padded neighbor rows and DVE reduce_max computes the segment max.

All index/permutation work happens on host at trace time (indices are
runtime inputs, but the Bass program is built inside kernel()).
"""
import numpy as np

import os

import concourse.bacc as bacc
import concourse.bass as bass
import concourse.mybir as mybir
import concourse.tile as tile
from concourse import masks as bass_masks
from concourse.bass_utils import run_bass_kernel_spmd

F32 = mybir.dt.float32
_ADDR_SPACE = os.environ.get("KERNEL_TABLE_SPACE", "Shared")
_NLAYERS = int(os.environ.get("KERNEL_NLAYERS", "23"))
I16 = mybir.dt.int16
NC = 8
N_ALL, N_PMT, N_VOX = 8600, 600, 8000
NBG = 9   # g-phase blocks per core (72 groups total, 68 real)
NBV = 8   # v-phase blocks per core (64 groups total, 63 real)


# ---------------------------------------------------------------- host prep

def _grouping(dst, n, nb_loc):
    """Degree-sort nodes into groups of 128; band groups by degree so core r,
    block j holds group band[j]*8 + r. Returns (new2old [NC*nb_loc*128],
    old2new [n], D [nb_loc] padded degree per block position, adj tokens)."""
    n_groups = NC * nb_loc
    deg = np.bincount(dst, minlength=n)
    order = np.argsort(deg, kind='stable')        # ascending degree, old ids
    slots = n_groups * 128
    new2old = np.full(slots, -1, dtype=np.int64)

    # group k (k < ceil(n/128)) = order[128k : 128k+128]; group Db
    real_groups = (n + 127) // 128
    gDb = np.ones(n_groups, dtype=np.int64)
    for k in range(real_groups):
        nodes = order[128 * k:128 * k + 128]
        gDb[k] = max(1, deg[nodes].max())
    # sort groups by Db desc; band j = ranks [8j, 8j+8); core r gets band[8j+r]
    grank = np.argsort(-gDb, kind='stable')
    D = np.zeros(nb_loc, dtype=np.int64)
    for j in range(nb_loc):
        band = grank[8 * j:8 * j + 8]
        D[j] = gDb[band].max()
        for r in range(8):
            k = band[r]
            base = NC if False else 0  # noqa
            nid0 = r * nb_loc * 128 + j * 128
            if k < real_groups:
                nodes = order[128 * k:128 * k + 128]
                new2old[nid0:nid0 + len(nodes)] = nodes
    old2new = np.full(n, -1, dtype=np.int64)
    valid = new2old >= 0
    old2new[new2old[valid]] = np.nonzero(valid)[0]
    return new2old, old2new, D


def _tables(src, dst, n, new2old, old2new, D, nb_loc):
    """Per-core int16 token arrays (concatenated per-block), packed 16-wise."""
    order = np.argsort(dst, kind='stable')
    s_sorted = src[order]
    deg = np.bincount(dst, minlength=n)
    starts = np.zeros(n + 1, dtype=np.int64)
    starts[1:] = np.cumsum(deg)
    idx_per_core = []
    for r in range(NC):
        toks = []
        for j in range(nb_loc):
            d = int(D[j])
            tok = np.zeros((d, 128), dtype=np.int16)
            for p in range(128):
                nid = r * nb_loc * 128 + j * 128 + p
                old = new2old[nid]
                if old >= 0:
                    di = int(deg[old])
                    nbrs = old2new[s_sorted[starts[old]:starts[old] + di]]
                    tok[:di, p] = nbrs
                    tok[di:, p] = nid
                # else leave 0 (dummy slot -> gathers row 0, output unused)
            toks.append(tok.reshape(-1))
        idx_per_core.append(np.concatenate(toks))
    return idx_per_core


def _pack16(v):
    """[T] int16 -> [128, T//16]: token t at partition t%16, col t//16,
    replicated across the 8 q7-core partition groups."""
    T = len(v)
    assert T % 16 == 0
    a = v.reshape(T // 16, 16).T.astype(np.int16)    # [16, T//16]
    return np.ascontiguousarray(np.tile(a, (8, 1)))


def _layer_descs():
    """23 layers: (wname, F_in, F_out, phase, mask_after, last)"""
    L = []
    for i in range(8):
        L.append(dict(F_in=54, F_out=54, phase='g'))
    for i in range(4):                      # a1..a4
        L.append(dict(F_in=54, F_out=54, phase='v'))
    L.append(dict(F_in=54, F_out=25, phase='v'))          # a5
    for i in range(9):                      # a6..a14
        L.append(dict(F_in=25, F_out=25, phase='v'))
    L.append(dict(F_in=25, F_out=1, phase='v'))           # a15
    L[10]['mask'] = 'u2'   # after a3
    L[13]['mask'] = 'u3'   # after a6
    L[16]['mask'] = 'u4'   # after a9
    L[22]['last'] = True
    return L


# ---------------------------------------------------------------- bass build

def _build(Dg, Dv, Tg, Tv, descs):
    nc = bacc.Bacc("TRN2", target_bir_lowering=False, debug=False, num_devices=NC)

    x0_in = nc.dram_tensor("x0", [128, NBG, 64], F32, kind="ExternalInput")
    w_in = nc.dram_tensor("w", [64, 46 * 64], F32, kind="ExternalInput")
    idxg_in = nc.dram_tensor("idxg", [128, Tg // 16], I16, kind="ExternalInput")
    idxv_in = nc.dram_tensor("idxv", [128, Tv // 16], I16, kind="ExternalInput")
    idxt_in = nc.dram_tensor("idxt", [128, 64], I16, kind="ExternalInput")
    mask_in = {k: nc.dram_tensor(k, [128, NBV, 64], F32, kind="ExternalInput")
               for k in ('u1', 'u2', 'u3', 'u4')}
    out_dram = nc.dram_tensor("out", [128, NBV], F32, kind="ExternalOutput")
    dbg_dram = (nc.dram_tensor("dbg", [128, NBG, 64], F32, kind="ExternalOutput")
                if _NLAYERS < 23 else None)

    with tile.TileContext(nc) as tc:
        with (
            tc.tile_pool(name="const", bufs=1) as cpool,
            tc.tile_pool(name="stage", bufs=2) as stage_pool,
            tc.tile_pool(name="outT", bufs=2) as outT_pool,
            tc.tile_pool(name="ys", bufs=2) as y_pool,
            tc.tile_pool(name="zs", bufs=2) as z_pool,
            tc.tile_pool(name="ms", bufs=2) as m_pool,
            tc.tile_pool(name="gath", bufs=6) as gath_pool,
            tc.tile_pool(name="fin", bufs=1) as fin_pool,
            tc.tile_pool(name="psum", bufs=1, space="PSUM") as psum_pool,
            tc.tile_pool(name="agin", bufs=2, space="DRAM") as agin_pool,
            tc.tile_pool(name="tables", bufs=2, space="DRAM") as table_pool,
        ):
            ident = cpool.tile([128, 128], F32)
            bass_masks.make_identity(nc, ident[:])
            w_sb = cpool.tile([64, 46 * 64], F32)
            nc.sync.dma_start(w_sb[:], w_in[:])
            idxg_sb = cpool.tile([128, Tg // 16], I16)
            nc.sync.dma_start(idxg_sb[:], idxg_in[:])
            idxv_sb = cpool.tile([128, Tv // 16], I16)
            nc.sync.dma_start(idxv_sb[:], idxv_in[:])
            idxt_sb = cpool.tile([128, 64], I16)
            nc.sync.dma_start(idxt_sb[:], idxt_in[:])
            mask_sb = {}
            for k, t in mask_in.items():
                mask_sb[k] = cpool.tile([128, NBV, 64], F32, name=f"mask_{k}")
                nc.sync.dma_start(mask_sb[k][:], t[:])

            stage = stage_pool.tile([128, NBG, 64], F32)
            nc.sync.dma_start(stage[:], x0_in[:])

            for l, L in enumerate(descs):
                Fi, Fo = L['F_in'], L['F_out']
                phase = L['phase']
                NB = NBG if phase == 'g' else NBV
                D = Dg if phase == 'g' else Dv
                idx_sb = idxg_sb if phase == 'g' else idxv_sb

                # --- transpose own stage -> feat-major outT
                outT = outT_pool.tile([64, NBG * 128], F32)
                for j in range(NB):
                    pt = psum_pool.tile([64, 128], F32, tag="pt", bufs=2)
                    nc.tensor.matmul(pt[:], stage[:, j, :], ident[:],
                                     is_transpose=True)
                    nc.vector.tensor_copy(outT[:, j * 128:(j + 1) * 128], pt[:])

                # --- y / z matmuls (own nodes)
                y_st = y_pool.tile([128, NBG, 64], F32)
                z_st = z_pool.tile([128, NBG, 64], F32)
                wt_ap = w_sb[:Fi, (2 * l) * 64:(2 * l) * 64 + 64]
                wp_ap = w_sb[:Fi + 1, (2 * l + 1) * 64:(2 * l + 1) * 64 + 64]
                for j in range(NB):
                    sl = slice(j * 128, (j + 1) * 128)
                    py = psum_pool.tile([128, 64], F32, tag="py", bufs=3)
                    nc.tensor.matmul(py[:], outT[:Fi, sl], wt_ap)
                    nc.vector.tensor_copy(y_st[:, j, :], py[:])
                    pz = psum_pool.tile([128, 64], F32, tag="pz", bufs=3)
                    nc.tensor.matmul(pz[:], outT[:Fi + 1, sl], wp_ap)
                    nc.vector.tensor_copy(z_st[:, j, :], pz[:])

                # --- AllGather y -> table
                ag_in = agin_pool.tile([NBG * 128, 64], F32)
                nc.sync.dma_start(
                    ag_in[:NB * 128].rearrange("(j p) f -> p j f", p=128),
                    y_st[:, :NB, :])
                table = table_pool.tile([NC * NB * 128, 64], F32,
                                        addr_space=_ADDR_SPACE, tag="table",
                                        name=f"table{l}")
                nc.gpsimd.collective_compute(
                    "AllGather", mybir.AluOpType.bypass,
                    replica_groups=[list(range(NC))],
                    ins=[ag_in[:NB * 128, :]], outs=[table[:]])

                # --- gather + blockwise segment max, pointwise per half so
                # half A's pointwise (and the next layer's transposes) overlap
                # half B's gathers/reduces on gpsimd.
                m_st = m_pool.tile([128, NBG, 64], F32)
                last = L.get('last')
                new_stage = (None if last else
                             stage_pool.tile([128, NBG, 64], F32, tag="stage"))

                def pointwise(jsl):
                    msl = m_st[:, jsl, :Fo]
                    nc.vector.tensor_max(msl, msl, y_st[:, jsl, :Fo])
                    nc.vector.tensor_sub(msl, msl, y_st[:, jsl, :Fo])
                    nsl = new_stage[:, jsl, :Fo]
                    nc.vector.tensor_add(nsl, msl, z_st[:, jsl, :Fo])
                    if L.get('mask'):
                        nc.vector.tensor_mul(
                            nsl, nsl, mask_sb[L['mask']][:, jsl, :Fo])
                    nc.vector.memset(new_stage[:, jsl, Fo:Fo + 1], 1.0)
                    if Fo + 1 < 64:
                        nc.vector.memset(new_stage[:, jsl, Fo + 1:], 0.0)

                half = NB // 2 + 1
                off = 0
                for j in range(NB):
                    d = int(D[j])
                    g_t = gath_pool.tile([128, d, 64], F32, tag="gath")
                    nc.gpsimd.dma_gather(
                        g_t[:], table[:], idx_sb[:, off:off + d * 8],
                        d * 128, d * 128, 64, single_packet=False)
                    off += d * 8
                    nc.vector.reduce_max(
                        m_st[:, j, :Fo],
                        g_t[:, :, :Fo].rearrange("p d f -> p f d"),
                        axis=mybir.AxisListType.X)
                    if not last and j == half - 1:
                        pointwise(slice(0, half))

                if last:
                    msl = m_st[:, :NB, :Fo]
                    nc.vector.tensor_max(msl, msl, y_st[:, :NB, :Fo])
                    nc.vector.tensor_sub(msl, msl, y_st[:, :NB, :Fo])
                    fin = fin_pool.tile([128, NBV], F32)
                    nc.vector.tensor_add(m_st[:, :NB, 0:1], msl,
                                         z_st[:, :NB, :Fo])
                    nc.vector.tensor_scalar_max(fin[:], m_st[:, :NB, 0], 0.0)
                    nc.sync.dma_start(out_dram[:], fin[:])
                    break
                pointwise(slice(half, NB))
                if NB < NBG:
                    nc.vector.memset(new_stage[:, NB:, :], 0.0)
                stage = new_stage

                if l == _NLAYERS - 1 and dbg_dram is not None:
                    nc.sync.dma_start(dbg_dram[:, :NB, :], stage[:, :NB, :])
                    break

                # --- transition after b8: reshard g-space out -> v-space
                if l == 7:
                    ag2 = agin_pool.tile([NBG * 128, 64], F32, tag="agin")
                    nc.sync.dma_start(
                        ag2[:].rearrange("(j p) f -> p j f", p=128),
                        stage[:, :, :])
                    ttable = table_pool.tile([NC * NBG * 128, 64], F32,
                                             addr_space=_ADDR_SPACE, tag="table",
                                             name="ttable")
                    nc.gpsimd.collective_compute(
                        "AllGather", mybir.AluOpType.bypass,
                        replica_groups=[list(range(NC))],
                        ins=[ag2[:, :]], outs=[ttable[:]])
                    g_t = gath_pool.tile([128, NBV, 64], F32, tag="gath")
                    nc.gpsimd.dma_gather(
                        g_t[:], ttable[:], idxt_sb[:, :NBV * 8],
                        NBV * 128, NBV * 128, 64, single_packet=False)
                    stage_v = stage_pool.tile([128, NBG, 64], F32, tag="stage")
                    nc.vector.tensor_mul(stage_v[:, :NBV, :], g_t[:],
                                         mask_sb['u1'][:])
                    stage = stage_v

    nc.compile()
    return nc




def _bench_pjrt(nc, in_maps, n_iter=10):
    """Repeat execution with device-resident inputs; report per-run wall times.

    Mirrors bass2jax.run_bass_via_pjrt's multi-core path but keeps inputs on
    device so repeated calls measure launch + execute (not input shipping)."""
    import time as _time

    import jax
    from jax.sharding import Mesh, PartitionSpec
    from jax.experimental.shard_map import shard_map

    import concourse.mybir as _mybir
    from concourse import bass2jax as b2j

    b2j.install_neuronx_cc_hook()
    partition_name = nc.partition_id_tensor.name if nc.partition_id_tensor else None
    in_names, out_names, out_avals, zero_outs = [], [], [], []
    for alloc in nc.m.functions[0].allocations:
        if not isinstance(alloc, _mybir.MemoryLocationSet):
            continue
        name = alloc.memorylocations[0].name
        if alloc.kind == "ExternalInput":
            if name != partition_name:
                in_names.append(name)
        elif alloc.kind == "ExternalOutput":
            shape = tuple(alloc.tensor_shape)
            dtype = _mybir.dt.np(alloc.dtype)
            out_names.append(name)
            out_avals.append(jax.core.ShapedArray(shape, dtype))
            zero_outs.append(np.zeros(shape, dtype))
    n_params = len(in_names)
    n_outs = len(out_avals)
    all_names = list(in_names) + out_names + ([partition_name] if partition_name else [])

    def _body(*args):
        operands = list(args)
        if partition_name is not None:
            operands.append(b2j.partition_id_tensor())
        return tuple(b2j._bass_exec_p.bind(
            *operands, out_avals=tuple(out_avals), in_names=tuple(all_names),
            out_names=tuple(out_names), lowering_input_output_aliases=(),
            sim_require_finite=True, sim_require_nnan=True, nc=nc))

    devices = jax.devices()[:NC]
    mesh = Mesh(np.asarray(devices), ("core",))
    sharded = jax.jit(
        shard_map(_body, mesh=mesh,
                  in_specs=(PartitionSpec("core"),) * (n_params + n_outs),
                  out_specs=(PartitionSpec("core"),) * n_outs,
                  check_rep=False),
        donate_argnums=tuple(range(n_params, n_params + n_outs)),
        keep_unused=True)
    sharding = jax.sharding.NamedSharding(mesh, PartitionSpec("core"))
    dev_in = [jax.device_put(
        np.concatenate([np.asarray(m[k]) for m in in_maps], axis=0), sharding)
        for k in in_names]
    times = []
    for i in range(n_iter):
        zeros = [jax.device_put(
            np.zeros((NC * z.shape[0], *z.shape[1:]), z.dtype), sharding)
            for z in zero_outs]
        for z in zeros:
            z.block_until_ready()
        t0 = _time.perf_counter()
        outs = sharded(*dev_in, *zeros)
        for o in outs:
            o.block_until_ready()
        times.append(_time.perf_counter() - t0)
    return times


# ---------------------------------------------------------------- kernel

def kernel(**inputs):
    f32 = np.float32
    inp = {k: np.asarray(v) for k, v in inputs.items()}

    x0 = np.concatenate([
        np.concatenate([inp['inputs'][0].astype(f32),
                        np.zeros((N_VOX, 51), f32)], axis=0),
        inp['koor'].astype(f32)], axis=1)                  # [8600, 54]
    um = {k: ((inp[k] > 0.5).astype(f32) * 2.0) for k in ('u1', 'u2', 'u3', 'u4')}

    # Self-edges only contribute y_own to the max; padding slots gather the
    # node's own row and an explicit max(m, y) covers full-degree blocks, so
    # drop them from the token tables entirely.
    gm = inp['src'] != inp['dst']
    vm = inp['vsrc'] != inp['vdst']
    g_src, g_dst = inp['src'][gm], inp['dst'][gm]
    v_src, v_dst = inp['vsrc'][vm], inp['vdst'][vm]
    g_n2o, g_o2n, Dg = _grouping(g_dst, N_ALL, NBG)
    v_n2o, v_o2n, Dv = _grouping(v_dst, N_VOX, NBV)
    idx_g = _tables(g_src, g_dst, N_ALL, g_n2o, g_o2n, Dg, NBG)
    idx_v = _tables(v_src, v_dst, N_VOX, v_n2o, v_o2n, Dv, NBV)
    Tg, Tv = 128 * int(Dg.sum()), 128 * int(Dv.sum())

    # per-core staged x0 (node-major, ones col at 54)
    x0_st = []
    for r in range(NC):
        st = np.zeros((128, NBG, 64), f32)
        st[:, :, 54] = 1.0
        for j in range(NBG):
            nid0 = r * NBG * 128 + j * 128
            olds = g_n2o[nid0:nid0 + 128]
            sel = olds >= 0
            st[sel, j, :54] = x0[olds[sel]]
        x0_st.append(st)

    # transition gather tokens: v-new-id -> g-new-id (of global old id 600+v)
    idxt = []
    for r in range(NC):
        tok = np.zeros(NBV * 128, np.int16)
        for j in range(NBV):
            for_p = v_n2o[r * NBV * 128 + j * 128: r * NBV * 128 + j * 128 + 128]
            t = np.zeros(128, np.int64)
            sel = for_p >= 0
            t[sel] = g_o2n[N_PMT + for_p[sel]]
            tok[j * 128:(j + 1) * 128] = t.astype(np.int16)
        idxt.append(_pack16(tok))

    # per-core masks (node-major padded)
    def mk_mask(u, F, ones_col):
        per = []
        for r in range(NC):
            mt = np.zeros((128, NBV, 64), f32)
            if ones_col is not None:
                mt[:, :, ones_col] = 1.0
            for j in range(NBV):
                olds = v_n2o[r * NBV * 128 + j * 128: r * NBV * 128 + j * 128 + 128]
                sel = olds >= 0
                mt[sel, j, :F] = u[olds[sel], :F]
            per.append(mt)
        return per
    m_u1 = mk_mask(um['u1'], 54, 54)
    m_u2 = mk_mask(um['u2'], 54, None)
    m_u3 = mk_mask(um['u3'], 25, None)
    m_u4 = mk_mask(um['u4'], 25, None)

    # weights: [64, 46*64]; layer l: wt at col 2l*64, wp_aug at (2l+1)*64
    descs = _layer_descs()
    wts = ([(inp['bwt'][i], inp['bbt'][i], inp['bwp'][i], inp['bbp'][i]) for i in range(8)]
           + [(inp['awt'][i], inp['abt'][i], inp['awp'][i], inp['abp'][i]) for i in range(4)]
           + [(inp['a5wt'], inp['a5bt'], inp['a5wp'], inp['a5bp'])]
           + [(inp['cwt'][i], inp['cbt'][i], inp['cwp'][i], inp['cbp'][i]) for i in range(9)]
           + [(inp['fwt'], inp['fbt'], inp['fwp'], inp['fbp'])])
    W = np.zeros((64, 46 * 64), f32)
    for l, (wt, bt, wp, bp) in enumerate(wts):
        Fi, Fo = wt.shape
        W[:Fi, 2 * l * 64:2 * l * 64 + Fo] = wt
        W[:Fi, (2 * l + 1) * 64:(2 * l + 1) * 64 + Fo] = wp
        W[Fi, (2 * l + 1) * 64:(2 * l + 1) * 64 + Fo] = bt + bp

    nc = _build(Dg, Dv, Tg, Tv, descs)

    in_maps = []
    for r in range(NC):
        in_maps.append({
            "x0": x0_st[r],
            "w": W,
            "idxg": _pack16(idx_g[r]),
            "idxv": _pack16(idx_v[r]),
            "idxt": idxt[r],
            "u1": m_u1[r], "u2": m_u2[r], "u3": m_u3[r], "u4": m_u4[r],
        })
    res = run_bass_kernel_spmd(nc, in_maps, core_ids=list(range(NC)))
    kernel.last_results = res
    nbench = int(os.environ.get("KERNEL_BENCH", "0"))
    if nbench:
        times = _bench_pjrt(nc, in_maps, nbench)
        kernel.bench_times = times
        print("bench ms:", " ".join(f"{t*1e3:.2f}" for t in times))

    out = np.zeros(N_VOX, f32)
    for r in range(NC):
        o = res.results[r]["out"]                  # [128, NBV]
        for j in range(NBV):
            olds = v_n2o[r * NBV * 128 + j * 128: r * NBV * 128 + j * 128 + 128]
            sel = olds >= 0
            out[olds[sel]] = o[sel, j]
    return out


if __name__ == "__main__":
    d = np.load('/tmp/inputs.npz')
    inputs = {k: d[k] for k in d.files}
    expected = np.load('/tmp/expected.npy')
    got = kernel(**inputs)
    rel = np.linalg.norm(got - expected) / np.linalg.norm(expected)
    print("rel_l2:", rel)



# revision 43
# speedup vs baseline: 1.0796x; 1.0443x over previous
"""Trainium2 Bass kernel for nn_Net_19619410608498 (EdgeConv GNN, 23 layers).

Algorithm (per EdgeConv layer, using max-commutation):
    y = x @ wt ;  z = x @ wp + (bt + bp)
    out = segment_max(y[src], dst) - y + z

Distribution: node-parallel across 8 cores. Nodes are degree-sorted into
groups of 128, groups banded by degree so every core's block-position j has
the SAME padded degree D(j) (identical SPMD shapes + perfect balance). Each
core computes y/z for its own nodes, an AllGather builds the full y-table in
DRAM (the AG output IS the gather table), then dma_gather fetches each
node's padded neighbor rows and DVE reduce_max computes the segment max.

All index/permutation work happens on host at trace time (indices are
runtime inputs, but the Bass program is built inside kernel()).
"""
import numpy as np

import os

import concourse.bacc as bacc
import concourse.bass as bass
import concourse.mybir as mybir
import concourse.tile as tile
from concourse import masks as bass_masks
from concourse.bass_utils import run_bass_kernel_spmd

F32 = mybir.dt.float32
F16 = mybir.dt.bfloat16
_ADDR_SPACE = os.environ.get("KERNEL_TABLE_SPACE", "Shared")
_NLAYERS = int(os.environ.get("KERNEL_NLAYERS", "23"))
I16 = mybir.dt.int16
NC = 8
N_ALL, N_PMT, N_VOX = 8600, 600, 8000
NBG = 9   # g-phase blocks per core (72 groups total, 68 real)
NBV = 8   # v-phase blocks per core (64 groups total, 63 real)


# ---------------------------------------------------------------- host prep

def _grouping(dst, n, nb_loc):
    """Degree-sort nodes into groups of 128; band groups by degree so core r,
    block j holds group band[j]*8 + r. Returns (new2old [NC*nb_loc*128],
    old2new [n], D [nb_loc] padded degree per block position, adj tokens)."""
    n_groups = NC * nb_loc
    deg = np.bincount(dst, minlength=n)
    order = np.argsort(deg, kind='stable')        # ascending degree, old ids
    slots = n_groups * 128
    new2old = np.full(slots, -1, dtype=np.int64)

    # group k (k < ceil(n/128)) = order[128k : 128k+128]; group Db
    real_groups = (n + 127) // 128
    gDb = np.ones(n_groups, dtype=np.int64)
    for k in range(real_groups):
        nodes = order[128 * k:128 * k + 128]
        gDb[k] = max(1, deg[nodes].max())
    # sort groups by Db desc; band j = ranks [8j, 8j+8); core r gets band[8j+r]
    grank = np.argsort(-gDb, kind='stable')
    D = np.zeros(nb_loc, dtype=np.int64)
    for j in range(nb_loc):
        band = grank[8 * j:8 * j + 8]
        D[j] = gDb[band].max()
        for r in range(8):
            k = band[r]
            base = NC if False else 0  # noqa
            nid0 = r * nb_loc * 128 + j * 128
            if k < real_groups:
                nodes = order[128 * k:128 * k + 128]
                new2old[nid0:nid0 + len(nodes)] = nodes
    old2new = np.full(n, -1, dtype=np.int64)
    valid = new2old >= 0
    old2new[new2old[valid]] = np.nonzero(valid)[0]
    return new2old, old2new, D


def _tables(src, dst, n, new2old, old2new, D, nb_loc):
    """Per-core int16 token arrays (concatenated per-block), packed 16-wise."""
    order = np.argsort(dst, kind='stable')
    s_sorted = src[order]
    deg = np.bincount(dst, minlength=n)
    starts = np.zeros(n + 1, dtype=np.int64)
    starts[1:] = np.cumsum(deg)
    idx_per_core = []
    for r in range(NC):
        toks = []
        for j in range(nb_loc):
            d = int(D[j])
            tok = np.zeros((d, 128), dtype=np.int16)
            for p in range(128):
                nid = r * nb_loc * 128 + j * 128 + p
                old = new2old[nid]
                if old >= 0:
                    di = int(deg[old])
                    nbrs = old2new[s_sorted[starts[old]:starts[old] + di]]
                    tok[:di, p] = nbrs
                    tok[di:, p] = nid
                # else leave 0 (dummy slot -> gathers row 0, output unused)
            toks.append(tok.reshape(-1))
        idx_per_core.append(np.concatenate(toks))
    return idx_per_core


def _pack16(v):
    """[T] int16 -> [128, T//16]: token t at partition t%16, col t//16,
    replicated across the 8 q7-core partition groups."""
    T = len(v)
    assert T % 16 == 0
    a = v.reshape(T // 16, 16).T.astype(np.int16)    # [16, T//16]
    return np.ascontiguousarray(np.tile(a, (8, 1)))


def _layer_descs():
    """23 layers: (wname, F_in, F_out, phase, mask_after, last)"""
    L = []
    for i in range(8):
        L.append(dict(F_in=54, F_out=54, phase='g'))
    for i in range(4):                      # a1..a4
        L.append(dict(F_in=54, F_out=54, phase='v'))
    L.append(dict(F_in=54, F_out=25, phase='v'))          # a5
    for i in range(9):                      # a6..a14
        L.append(dict(F_in=25, F_out=25, phase='v'))
    L.append(dict(F_in=25, F_out=1, phase='v'))           # a15
    L[10]['mask'] = 'u2'   # after a3
    L[13]['mask'] = 'u3'   # after a6
    L[16]['mask'] = 'u4'   # after a9
    L[22]['last'] = True
    return L


# ---------------------------------------------------------------- bass build

def _build(Dg, Dv, Tg, Tv, descs):
    nc = bacc.Bacc("TRN2", target_bir_lowering=False, debug=False, num_devices=NC)

    x0_in = nc.dram_tensor("x0", [128, NBG, 64], F32, kind="ExternalInput")
    w_in = nc.dram_tensor("w", [64, 46 * 64], F32, kind="ExternalInput")
    idxg_in = nc.dram_tensor("idxg", [128, Tg // 16], I16, kind="ExternalInput")
    idxv_in = nc.dram_tensor("idxv", [128, Tv // 16], I16, kind="ExternalInput")
    idxt_in = nc.dram_tensor("idxt", [128, 64], I16, kind="ExternalInput")
    mask_in = {k: nc.dram_tensor(k, [128, NBV, 64], F32, kind="ExternalInput")
               for k in ('u1', 'u2', 'u3', 'u4')}
    out_dram = nc.dram_tensor("out", [128, NBV], F32, kind="ExternalOutput")
    dbg_dram = (nc.dram_tensor("dbg", [128, NBG, 64], F32, kind="ExternalOutput")
                if _NLAYERS < 23 else None)

    with tile.TileContext(nc) as tc:
        with (
            tc.tile_pool(name="const", bufs=1) as cpool,
            tc.tile_pool(name="stage", bufs=2) as stage_pool,
            tc.tile_pool(name="outT", bufs=2) as outT_pool,
            tc.tile_pool(name="ys", bufs=2) as y_pool,
            tc.tile_pool(name="zs", bufs=2) as z_pool,
            tc.tile_pool(name="ms", bufs=2) as m_pool,
            tc.tile_pool(name="gath", bufs=4) as gath_pool,
            tc.tile_pool(name="fin", bufs=1) as fin_pool,
            tc.tile_pool(name="psum", bufs=1, space="PSUM") as psum_pool,
            tc.tile_pool(name="agin", bufs=2, space="DRAM") as agin_pool,
            tc.tile_pool(name="tables", bufs=2, space="DRAM") as table_pool,
        ):
            ident = cpool.tile([128, 128], F32)
            bass_masks.make_identity(nc, ident[:])
            ident16 = cpool.tile([128, 128], F16)
            nc.vector.tensor_copy(ident16[:], ident[:])
            # SBUF-resident f16 gather table: node nid at partition nid%128,
            # 256B stripe nid//128 (f16[:64] = features, rest zero pad).
            # Double-buffered by layer parity: the gather DMAs read the table
            # asynchronously after desc-gen, so a single buffer would let the
            # next layer's table copy race in-flight reads.
            tabs = []
            for h in range(2):
                t = cpool.tile([128, NC * NBG, 128], F16, name=f"tab_sb{h}")
                nc.vector.memset(t[:], 0.0)
                tabs.append(t)
            w_sb = cpool.tile([64, 46 * 64], F32)
            nc.sync.dma_start(w_sb[:], w_in[:])
            idxg_sb = cpool.tile([128, Tg // 16], I16)
            nc.sync.dma_start(idxg_sb[:], idxg_in[:])
            idxv_sb = cpool.tile([128, Tv // 16], I16)
            nc.sync.dma_start(idxv_sb[:], idxv_in[:])
            idxt_sb = cpool.tile([128, 64], I16)
            nc.sync.dma_start(idxt_sb[:], idxt_in[:])
            mask_sb = {}
            for k, t in mask_in.items():
                mask_sb[k] = cpool.tile([128, NBV, 64], F32, name=f"mask_{k}")
                nc.sync.dma_start(mask_sb[k][:], t[:])

            stage = stage_pool.tile([128, NBG, 64], F32)
            nc.sync.dma_start(stage[:], x0_in[:])

            for l, L in enumerate(descs):
                Fi, Fo = L['F_in'], L['F_out']
                phase = L['phase']
                NB = NBG if phase == 'g' else NBV
                D = Dg if phase == 'g' else Dv
                idx_sb = idxg_sb if phase == 'g' else idxv_sb

                # --- transpose own stage -> feat-major outT
                outT = outT_pool.tile([64, NBG * 128], F32)
                for j in range(NB):
                    pt = psum_pool.tile([64, 128], F32, tag="pt", bufs=2)
                    nc.tensor.matmul(pt[:], stage[:, j, :], ident[:],
                                     is_transpose=True)
                    nc.vector.tensor_copy(outT[:, j * 128:(j + 1) * 128], pt[:])

                # --- y / z matmuls (own nodes)
                y_st = y_pool.tile([128, NBG, 64], F32)
                y16 = y_pool.tile([128, NBG, 64], F16, tag="y16")
                z_st = z_pool.tile([128, NBG, 64], F32)
                wt_ap = w_sb[:Fi, (2 * l) * 64:(2 * l) * 64 + 64]
                wp_ap = w_sb[:Fi + 1, (2 * l + 1) * 64:(2 * l + 1) * 64 + 64]
                for j in range(NB):
                    sl = slice(j * 128, (j + 1) * 128)
                    py = psum_pool.tile([128, 64], F32, tag="py", bufs=2)
                    nc.tensor.matmul(py[:], outT[:Fi, sl], wt_ap)
                    nc.vector.tensor_copy(y_st[:, j, :], py[:])
                    pz = psum_pool.tile([128, 64], F32, tag="pz", bufs=2)
                    nc.tensor.matmul(pz[:], outT[:Fi + 1, sl], wp_ap)
                    nc.vector.tensor_copy(z_st[:, j, :], pz[:])

                # --- AllGather y (f32) -> DRAM -> SBUF -> cast to f16 table
                ag_in = agin_pool.tile([NBG * 128, 64], F32)
                nc.sync.dma_start(
                    ag_in[:NB * 128].rearrange("(j p) f -> p j f", p=128),
                    y_st[:, :NB, :])
                table = table_pool.tile([NC * NB * 128, 64], F32,
                                        addr_space=_ADDR_SPACE, tag="table",
                                        name=f"table{l}")
                nc.gpsimd.collective_compute(
                    "AllGather", mybir.AluOpType.bypass,
                    replica_groups=[list(range(NC))],
                    ins=[ag_in[:NB * 128, :]], outs=[table[:]])
                tstg = m_pool.tile([128, NC * NBG, 64], F32, tag="tstg")
                nc.sync.dma_start(
                    tstg[:, :NC * NB, :],
                    table[:].rearrange("(j p) f -> p j f", p=128))
                tab_sb = tabs[l % 2]
                nc.vector.tensor_copy(tab_sb[:, :NC * NB, :64],
                                      tstg[:, :NC * NB, :])

                # --- gather + blockwise segment max, pointwise per half so
                # half A's pointwise (and the next layer's transposes) overlap
                # half B's gathers/reduces on gpsimd.
                m_st = m_pool.tile([128, NBG, 64], F32)
                last = L.get('last')
                new_stage = (None if last else
                             stage_pool.tile([128, NBG, 64], F32, tag="stage"))

                def pointwise(jsl):
                    msl = m_st[:, jsl, :Fo]
                    nc.vector.tensor_max(msl, msl, y_st[:, jsl, :Fo])
                    nc.vector.tensor_sub(msl, msl, y_st[:, jsl, :Fo])
                    nsl = new_stage[:, jsl, :Fo]
                    nc.vector.tensor_add(nsl, msl, z_st[:, jsl, :Fo])
                    if L.get('mask'):
                        nc.vector.tensor_mul(
                            nsl, nsl, mask_sb[L['mask']][:, jsl, :Fo])
                    nc.vector.memset(new_stage[:, jsl, Fo:Fo + 1], 1.0)
                    if Fo + 1 < 64:
                        nc.vector.memset(new_stage[:, jsl, Fo + 1:], 0.0)

                half = NB // 2 + 1
                off = 0
                for j in range(NB):
                    d = int(D[j])
                    g_t = gath_pool.tile([128, 1, d * 128], F16, tag="gath")
                    nc.gpsimd.dma_gather(
                        g_t[:], tab_sb[:].rearrange("p j f -> p (j f)"),
                        idx_sb[:, off:off + d * 8],
                        d * 128, d * 128, 128, single_packet=False,
                        transpose=True, sbuf_tokens_per_rank=128,
                        sbuf_free_dim_per_rank=256)
                    off += d * 8
                    # feat-major max over the d stride-128 token slots
                    m16 = m_pool.tile([128, 128], F16, tag="m16")
                    nc.vector.reduce_max(
                        m16[:],
                        g_t[:, 0, :].rearrange("q (d p) -> q p d", d=d),
                        axis=mybir.AxisListType.X)
                    # transpose back to node-major and widen to f32
                    pmt = psum_pool.tile([128, 128], F16, tag="pmt", bufs=2)
                    nc.tensor.matmul(pmt[:], m16[:], ident16[:],
                                     is_transpose=True)
                    nc.vector.tensor_copy(m_st[:, j, :Fo], pmt[:, :Fo])
                    if not last and j == half - 1:
                        pointwise(slice(0, half))

                if last:
                    msl = m_st[:, :NB, :Fo]
                    nc.vector.tensor_max(msl, msl, y_st[:, :NB, :Fo])
                    nc.vector.tensor_sub(msl, msl, y_st[:, :NB, :Fo])
                    fin = fin_pool.tile([128, NBV], F32)
                    nc.vector.tensor_add(m_st[:, :NB, 0:1], msl,
                                         z_st[:, :NB, :Fo])
                    nc.vector.tensor_scalar_max(fin[:], m_st[:, :NB, 0], 0.0)
                    nc.sync.dma_start(out_dram[:], fin[:])
                    break
                pointwise(slice(half, NB))
                if NB < NBG:
                    nc.vector.memset(new_stage[:, NB:, :], 0.0)
                stage = new_stage

                if l == _NLAYERS - 1 and dbg_dram is not None:
                    nc.sync.dma_start(dbg_dram[:, :NB, :], stage[:, :NB, :])
                    break

                # --- transition after b8: reshard g-space out -> v-space
                if l == 7:
                    ag2 = agin_pool.tile([NBG * 128, 64], F32, tag="agin")
                    nc.sync.dma_start(
                        ag2[:].rearrange("(j p) f -> p j f", p=128),
                        stage[:, :, :])
                    ttable = table_pool.tile([NC * NBG * 128, 64], F32,
                                             addr_space=_ADDR_SPACE,
                                             tag="ttable", name="ttable")
                    nc.gpsimd.collective_compute(
                        "AllGather", mybir.AluOpType.bypass,
                        replica_groups=[list(range(NC))],
                        ins=[ag2[:, :]], outs=[ttable[:]])
                    g_t = gath_pool.tile([128, NBV, 64], F32, tag="gath")
                    nc.gpsimd.dma_gather(
                        g_t[:], ttable[:], idxt_sb[:, :NBV * 8],
                        NBV * 128, NBV * 128, 64, single_packet=False)
                    stage_v = stage_pool.tile([128, NBG, 64], F32, tag="stage")
                    nc.vector.tensor_mul(stage_v[:, :NBV, :], g_t[:],
                                         mask_sb['u1'][:])
                    stage = stage_v

    nc.compile()
    return nc




def _bench_pjrt(nc, in_maps, n_iter=10):
    """Repeat execution with device-resident inputs; report per-run wall times.

    Mirrors bass2jax.run_bass_via_pjrt's multi-core path but keeps inputs on
    device so repeated calls measure launch + execute (not input shipping)."""
    import time as _time

    import jax
    from jax.sharding import Mesh, PartitionSpec
    from jax.experimental.shard_map import shard_map

    import concourse.mybir as _mybir
    from concourse import bass2jax as b2j

    b2j.install_neuronx_cc_hook()
    partition_name = nc.partition_id_tensor.name if nc.partition_id_tensor else None
    in_names, out_names, out_avals, zero_outs = [], [], [], []
    for alloc in nc.m.functions[0].allocations:
        if not isinstance(alloc, _mybir.MemoryLocationSet):
            continue
        name = alloc.memorylocations[0].name
        if alloc.kind == "ExternalInput":
            if name != partition_name:
                in_names.append(name)
        elif alloc.kind == "ExternalOutput":
            shape = tuple(alloc.tensor_shape)
            dtype = _mybir.dt.np(alloc.dtype)
            out_names.append(name)
            out_avals.append(jax.core.ShapedArray(shape, dtype))
            zero_outs.append(np.zeros(shape, dtype))
    n_params = len(in_names)
    n_outs = len(out_avals)
    all_names = list(in_names) + out_names + ([partition_name] if partition_name else [])

    def _body(*args):
        operands = list(args)
        if partition_name is not None:
            operands.append(b2j.partition_id_tensor())
        return tuple(b2j._bass_exec_p.bind(
            *operands, out_avals=tuple(out_avals), in_names=tuple(all_names),
            out_names=tuple(out_names), lowering_input_output_aliases=(),
            sim_require_finite=True, sim_require_nnan=True, nc=nc))

    devices = jax.devices()[:NC]
    mesh = Mesh(np.asarray(devices), ("core",))
    sharded = jax.jit(
        shard_map(_body, mesh=mesh,
                  in_specs=(PartitionSpec("core"),) * (n_params + n_outs),
                  out_specs=(PartitionSpec("core"),) * n_outs,
                  check_rep=False),
        donate_argnums=tuple(range(n_params, n_params + n_outs)),
        keep_unused=True)
    sharding = jax.sharding.NamedSharding(mesh, PartitionSpec("core"))
    dev_in = [jax.device_put(
        np.concatenate([np.asarray(m[k]) for m in in_maps], axis=0), sharding)
        for k in in_names]
    times = []
    for i in range(n_iter):
        zeros = [jax.device_put(
            np.zeros((NC * z.shape[0], *z.shape[1:]), z.dtype), sharding)
            for z in zero_outs]
        for z in zeros:
            z.block_until_ready()
        t0 = _time.perf_counter()
        outs = sharded(*dev_in, *zeros)
        for o in outs:
            o.block_until_ready()
        times.append(_time.perf_counter() - t0)
    return times


# ---------------------------------------------------------------- kernel

def kernel(**inputs):
    f32 = np.float32
    inp = {k: np.asarray(v) for k, v in inputs.items()}

    x0 = np.concatenate([
        np.concatenate([inp['inputs'][0].astype(f32),
                        np.zeros((N_VOX, 51), f32)], axis=0),
        inp['koor'].astype(f32)], axis=1)                  # [8600, 54]
    um = {k: ((inp[k] > 0.5).astype(f32) * 2.0) for k in ('u1', 'u2', 'u3', 'u4')}

    # Self-edges only contribute y_own to the max; padding slots gather the
    # node's own row and an explicit max(m, y) covers full-degree blocks, so
    # drop them from the token tables entirely.
    gm = inp['src'] != inp['dst']
    vm = inp['vsrc'] != inp['vdst']
    g_src, g_dst = inp['src'][gm], inp['dst'][gm]
    v_src, v_dst = inp['vsrc'][vm], inp['vdst'][vm]
    g_n2o, g_o2n, Dg = _grouping(g_dst, N_ALL, NBG)
    v_n2o, v_o2n, Dv = _grouping(v_dst, N_VOX, NBV)
    idx_g = _tables(g_src, g_dst, N_ALL, g_n2o, g_o2n, Dg, NBG)
    idx_v = _tables(v_src, v_dst, N_VOX, v_n2o, v_o2n, Dv, NBV)
    Tg, Tv = 128 * int(Dg.sum()), 128 * int(Dv.sum())

    # per-core staged x0 (node-major, ones col at 54)
    x0_st = []
    for r in range(NC):
        st = np.zeros((128, NBG, 64), f32)
        st[:, :, 54] = 1.0
        for j in range(NBG):
            nid0 = r * NBG * 128 + j * 128
            olds = g_n2o[nid0:nid0 + 128]
            sel = olds >= 0
            st[sel, j, :54] = x0[olds[sel]]
        x0_st.append(st)

    # transition gather tokens: v-new-id -> g-new-id (of global old id 600+v)
    idxt = []
    for r in range(NC):
        tok = np.zeros(NBV * 128, np.int16)
        for j in range(NBV):
            for_p = v_n2o[r * NBV * 128 + j * 128: r * NBV * 128 + j * 128 + 128]
            t = np.zeros(128, np.int64)
            sel = for_p >= 0
            t[sel] = g_o2n[N_PMT + for_p[sel]]
            tok[j * 128:(j + 1) * 128] = t.astype(np.int16)
        idxt.append(_pack16(tok))

    # per-core masks (node-major padded)
    def mk_mask(u, F, ones_col):
        per = []
        for r in range(NC):
            mt = np.zeros((128, NBV, 64), f32)
            if ones_col is not None:
                mt[:, :, ones_col] = 1.0
            for j in range(NBV):
                olds = v_n2o[r * NBV * 128 + j * 128: r * NBV * 128 + j * 128 + 128]
                sel = olds >= 0
                mt[sel, j, :F] = u[olds[sel], :F]
            per.append(mt)
        return per
    m_u1 = mk_mask(um['u1'], 54, 54)
    m_u2 = mk_mask(um['u2'], 54, None)
    m_u3 = mk_mask(um['u3'], 25, None)
    m_u4 = mk_mask(um['u4'], 25, None)

    # weights: [64, 46*64]; layer l: wt at col 2l*64, wp_aug at (2l+1)*64
    descs = _layer_descs()
    wts = ([(inp['bwt'][i], inp['bbt'][i], inp['bwp'][i], inp['bbp'][i]) for i in range(8)]
           + [(inp['awt'][i], inp['abt'][i], inp['awp'][i], inp['abp'][i]) for i in range(4)]
           + [(inp['a5wt'], inp['a5bt'], inp['a5wp'], inp['a5bp'])]
           + [(inp['cwt'][i], inp['cbt'][i], inp['cwp'][i], inp['cbp'][i]) for i in range(9)]
           + [(inp['fwt'], inp['fbt'], inp['fwp'], inp['fbp'])])
    W = np.zeros((64, 46 * 64), f32)
    for l, (wt, bt, wp, bp) in enumerate(wts):
        Fi, Fo = wt.shape
        W[:Fi, 2 * l * 64:2 * l * 64 + Fo] = wt
        W[:Fi, (2 * l + 1) * 64:(2 * l + 1) * 64 + Fo] = wp
        W[Fi, (2 * l + 1) * 64:(2 * l + 1) * 64 + Fo] = bt + bp

    nc = _build(Dg, Dv, Tg, Tv, descs)

    in_maps = []
    for r in range(NC):
        in_maps.append({
            "x0": x0_st[r],
            "w": W,
            "idxg": _pack16(idx_g[r]),
            "idxv": _pack16(idx_v[r]),
            "idxt": idxt[r],
            "u1": m_u1[r], "u2": m_u2[r], "u3": m_u3[r], "u4": m_u4[r],
        })
    res = run_bass_kernel_spmd(nc, in_maps, core_ids=list(range(NC)))
    kernel.last_results = res
    nbench = int(os.environ.get("KERNEL_BENCH", "0"))
    if nbench:
        times = _bench_pjrt(nc, in_maps, nbench)
        kernel.bench_times = times
        print("bench ms:", " ".join(f"{t*1e3:.2f}" for t in times))

    out = np.zeros(N_VOX, f32)
    for r in range(NC):
        o = res.results[r]["out"]                  # [128, NBV]
        for j in range(NBV):
            olds = v_n2o[r * NBV * 128 + j * 128: r * NBV * 128 + j * 128 + 128]
            sel = olds >= 0
            out[olds[sel]] = o[sel, j]
    return out


if __name__ == "__main__":
    d = np.load('/tmp/inputs.npz')
    inputs = {k: d[k] for k in d.files}
    expected = np.load('/tmp/expected.npy')
    got = kernel(**inputs)
    rel = np.linalg.norm(got - expected) / np.linalg.norm(expected)
    print("rel_l2:", rel)



# revision 45
# speedup vs baseline: 1.1032x; 1.0218x over previous
"""Trainium2 Bass kernel for nn_Net_19619410608498 (EdgeConv GNN, 23 layers).

Algorithm (per EdgeConv layer, using max-commutation):
    y = x @ wt ;  z = x @ wp + (bt + bp)
    out = segment_max(y[src], dst) - y + z

Distribution: node-parallel across 8 cores. Nodes are degree-sorted into
groups of 128, groups banded by degree so every core's block-position j has
the SAME padded degree D(j) (identical SPMD shapes + perfect balance). Each
core computes y/z for its own nodes, an AllGather builds the full y-table in
DRAM (the AG output IS the gather table), then dma_gather fetches each
node's padded neighbor rows and DVE reduce_max computes the segment max.

All index/permutation work happens on host at trace time (indices are
runtime inputs, but the Bass program is built inside kernel()).
"""
import numpy as np

import os

import concourse.bacc as bacc
import concourse.bass as bass
import concourse.mybir as mybir
import concourse.tile as tile
from concourse import masks as bass_masks
from concourse.bass_utils import run_bass_kernel_spmd

F32 = mybir.dt.float32
F16 = mybir.dt.bfloat16
_ADDR_SPACE = os.environ.get("KERNEL_TABLE_SPACE", "Shared")
_NLAYERS = int(os.environ.get("KERNEL_NLAYERS", "23"))
I16 = mybir.dt.int16
NC = 8
N_ALL, N_PMT, N_VOX = 8600, 600, 8000
NBG = 9   # g-phase blocks per core (72 groups total, 68 real)
NBV = 8   # v-phase blocks per core (64 groups total, 63 real)


# ---------------------------------------------------------------- host prep

def _grouping(dst, n, nb_loc):
    """Degree-sort nodes into groups of 128; band groups by degree so core r,
    block j holds group band[j]*8 + r. Returns (new2old [NC*nb_loc*128],
    old2new [n], D [nb_loc] padded degree per block position, adj tokens)."""
    n_groups = NC * nb_loc
    deg = np.bincount(dst, minlength=n)
    order = np.argsort(deg, kind='stable')        # ascending degree, old ids
    slots = n_groups * 128
    new2old = np.full(slots, -1, dtype=np.int64)

    # group k (k < ceil(n/128)) = order[128k : 128k+128]; group Db
    real_groups = (n + 127) // 128
    gDb = np.ones(n_groups, dtype=np.int64)
    for k in range(real_groups):
        nodes = order[128 * k:128 * k + 128]
        gDb[k] = max(1, deg[nodes].max())
    # sort groups by Db desc; band j = ranks [8j, 8j+8); core r gets band[8j+r]
    grank = np.argsort(-gDb, kind='stable')
    D = np.zeros(nb_loc, dtype=np.int64)
    for j in range(nb_loc):
        band = grank[8 * j:8 * j + 8]
        D[j] = gDb[band].max()
        for r in range(8):
            k = band[r]
            base = NC if False else 0  # noqa
            nid0 = r * nb_loc * 128 + j * 128
            if k < real_groups:
                nodes = order[128 * k:128 * k + 128]
                new2old[nid0:nid0 + len(nodes)] = nodes
    old2new = np.full(n, -1, dtype=np.int64)
    valid = new2old >= 0
    old2new[new2old[valid]] = np.nonzero(valid)[0]
    return new2old, old2new, D


def _tables(src, dst, n, new2old, old2new, D, nb_loc):
    """Per-core int16 token arrays (concatenated per-block), packed 16-wise."""
    order = np.argsort(dst, kind='stable')
    s_sorted = src[order]
    deg = np.bincount(dst, minlength=n)
    starts = np.zeros(n + 1, dtype=np.int64)
    starts[1:] = np.cumsum(deg)
    idx_per_core = []
    for r in range(NC):
        toks = []
        for j in range(nb_loc):
            d = int(D[j])
            tok = np.zeros((d, 128), dtype=np.int16)
            for p in range(128):
                nid = r * nb_loc * 128 + j * 128 + p
                old = new2old[nid]
                if old >= 0:
                    di = int(deg[old])
                    nbrs = old2new[s_sorted[starts[old]:starts[old] + di]]
                    tok[:di, p] = nbrs
                    tok[di:, p] = nid
                # else leave 0 (dummy slot -> gathers row 0, output unused)
            toks.append(tok.reshape(-1))
        idx_per_core.append(np.concatenate(toks))
    return idx_per_core


def _pack16(v):
    """[T] int16 -> [128, T//16]: token t at partition t%16, col t//16,
    replicated across the 8 q7-core partition groups."""
    T = len(v)
    assert T % 16 == 0
    a = v.reshape(T // 16, 16).T.astype(np.int16)    # [16, T//16]
    return np.ascontiguousarray(np.tile(a, (8, 1)))


def _layer_descs():
    """23 layers: (wname, F_in, F_out, phase, mask_after, last)"""
    L = []
    for i in range(8):
        L.append(dict(F_in=54, F_out=54, phase='g'))
    for i in range(4):                      # a1..a4
        L.append(dict(F_in=54, F_out=54, phase='v'))
    L.append(dict(F_in=54, F_out=25, phase='v'))          # a5
    for i in range(9):                      # a6..a14
        L.append(dict(F_in=25, F_out=25, phase='v'))
    L.append(dict(F_in=25, F_out=1, phase='v'))           # a15
    L[10]['mask'] = 'u2'   # after a3
    L[13]['mask'] = 'u3'   # after a6
    L[16]['mask'] = 'u4'   # after a9
    L[22]['last'] = True
    return L


# ---------------------------------------------------------------- bass build

def _build(Dg, Dv, Tg, Tv, descs):
    nc = bacc.Bacc("TRN2", target_bir_lowering=False, debug=False, num_devices=NC)

    x0_in = nc.dram_tensor("x0", [128, NBG, 64], F32, kind="ExternalInput")
    w_in = nc.dram_tensor("w", [64, 46 * 64], F32, kind="ExternalInput")
    idxg_in = nc.dram_tensor("idxg", [128, Tg // 16], I16, kind="ExternalInput")
    idxv_in = nc.dram_tensor("idxv", [128, Tv // 16], I16, kind="ExternalInput")
    idxt_in = nc.dram_tensor("idxt", [128, 64], I16, kind="ExternalInput")
    mask_in = {k: nc.dram_tensor(k, [128, NBV, 64], F32, kind="ExternalInput")
               for k in ('u1', 'u2', 'u3', 'u4')}
    out_dram = nc.dram_tensor("out", [128, NBV], F32, kind="ExternalOutput")
    dbg_dram = (nc.dram_tensor("dbg", [128, NBG, 64], F32, kind="ExternalOutput")
                if _NLAYERS < 23 else None)

    with tile.TileContext(nc) as tc:
        with (
            tc.tile_pool(name="const", bufs=1) as cpool,
            tc.tile_pool(name="stage", bufs=2) as stage_pool,
            tc.tile_pool(name="outT", bufs=2) as outT_pool,
            tc.tile_pool(name="ys", bufs=2) as y_pool,
            tc.tile_pool(name="zs", bufs=2) as z_pool,
            tc.tile_pool(name="ms", bufs=2) as m_pool,
            tc.tile_pool(name="gath", bufs=4) as gath_pool,
            tc.tile_pool(name="fin", bufs=1) as fin_pool,
            tc.tile_pool(name="psum", bufs=1, space="PSUM") as psum_pool,
            tc.tile_pool(name="agin", bufs=2, space="DRAM") as agin_pool,
            tc.tile_pool(name="tables", bufs=2, space="DRAM") as table_pool,
        ):
            ident = cpool.tile([128, 128], F32)
            bass_masks.make_identity(nc, ident[:])
            ident16 = cpool.tile([128, 128], F16)
            nc.vector.tensor_copy(ident16[:], ident[:])
            # SBUF-resident f16 gather table: node nid at partition nid%128,
            # 256B stripe nid//128 (f16[:64] = features, rest zero pad).
            # Double-buffered by layer parity: the gather DMAs read the table
            # asynchronously after desc-gen, so a single buffer would let the
            # next layer's table copy race in-flight reads.
            tabs = []
            for h in range(2):
                t = cpool.tile([128, NC * NBG, 128], F16, name=f"tab_sb{h}")
                nc.vector.memset(t[:], 0.0)
                tabs.append(t)
            w_sb = cpool.tile([64, 46 * 64], F32)
            nc.sync.dma_start(w_sb[:], w_in[:])
            idxg_sb = cpool.tile([128, Tg // 16], I16)
            nc.sync.dma_start(idxg_sb[:], idxg_in[:])
            idxv_sb = cpool.tile([128, Tv // 16], I16)
            nc.sync.dma_start(idxv_sb[:], idxv_in[:])
            idxt_sb = cpool.tile([128, 64], I16)
            nc.sync.dma_start(idxt_sb[:], idxt_in[:])
            mask_sb = {}
            for k, t in mask_in.items():
                mask_sb[k] = cpool.tile([128, NBV, 64], F32, name=f"mask_{k}")
                nc.sync.dma_start(mask_sb[k][:], t[:])

            stage = stage_pool.tile([128, NBG, 64], F32)
            nc.sync.dma_start(stage[:], x0_in[:])

            for l, L in enumerate(descs):
                Fi, Fo = L['F_in'], L['F_out']
                phase = L['phase']
                NB = NBG if phase == 'g' else NBV
                D = Dg if phase == 'g' else Dv
                idx_sb = idxg_sb if phase == 'g' else idxv_sb

                # --- transpose own stage -> feat-major outT
                outT = outT_pool.tile([64, NBG * 128], F32)
                for j in range(NB):
                    pt = psum_pool.tile([64, 128], F32, tag="pt", bufs=2)
                    nc.tensor.matmul(pt[:], stage[:, j, :], ident[:],
                                     is_transpose=True)
                    nc.vector.tensor_copy(outT[:, j * 128:(j + 1) * 128], pt[:])

                # --- y / z matmuls (own nodes)
                y_st = y_pool.tile([128, NBG, 64], F32)
                z_st = z_pool.tile([128, NBG, 64], F32)
                wt_ap = w_sb[:Fi, (2 * l) * 64:(2 * l) * 64 + 64]
                wp_ap = w_sb[:Fi + 1, (2 * l + 1) * 64:(2 * l + 1) * 64 + 64]
                for j in range(NB):
                    sl = slice(j * 128, (j + 1) * 128)
                    py = psum_pool.tile([128, 64], F32, tag="py", bufs=2)
                    nc.tensor.matmul(py[:], outT[:Fi, sl], wt_ap)
                    nc.vector.tensor_copy(y_st[:, j, :], py[:])
                    pz = psum_pool.tile([128, 64], F32, tag="pz", bufs=2)
                    nc.tensor.matmul(pz[:], outT[:Fi + 1, sl], wp_ap)
                    nc.vector.tensor_copy(z_st[:, j, :], pz[:])

                # --- cast y to bf16, AllGather bf16, DMA into the SBUF table
                y16 = y_pool.tile([128, NBG, 64], F16, tag="y16")
                nc.vector.tensor_copy(y16[:, :NB, :], y_st[:, :NB, :])
                ag_in = agin_pool.tile([NBG * 128, 64], F16)
                nc.sync.dma_start(
                    ag_in[:NB * 128].rearrange("(j p) f -> p j f", p=128),
                    y16[:, :NB, :])
                table = table_pool.tile([NC * NB * 128, 64], F16,
                                        addr_space=_ADDR_SPACE, tag="table",
                                        name=f"table{l}")
                nc.gpsimd.collective_compute(
                    "AllGather", mybir.AluOpType.bypass,
                    replica_groups=[list(range(NC))],
                    ins=[ag_in[:NB * 128, :]], outs=[table[:]])
                tab_sb = tabs[l % 2]
                nc.sync.dma_start(
                    tab_sb[:, :NC * NB, :64],
                    table[:].rearrange("(j p) f -> p j f", p=128))

                # --- gather + blockwise segment max, pointwise per half so
                # half A's pointwise (and the next layer's transposes) overlap
                # half B's gathers/reduces on gpsimd.
                m_st = m_pool.tile([128, NBG, 64], F32)
                last = L.get('last')
                new_stage = (None if last else
                             stage_pool.tile([128, NBG, 64], F32, tag="stage"))

                def pointwise(jsl):
                    msl = m_st[:, jsl, :Fo]
                    nc.vector.tensor_max(msl, msl, y_st[:, jsl, :Fo])
                    nc.vector.tensor_sub(msl, msl, y_st[:, jsl, :Fo])
                    nsl = new_stage[:, jsl, :Fo]
                    nc.vector.tensor_add(nsl, msl, z_st[:, jsl, :Fo])
                    if L.get('mask'):
                        nc.vector.tensor_mul(
                            nsl, nsl, mask_sb[L['mask']][:, jsl, :Fo])
                    nc.vector.memset(new_stage[:, jsl, Fo:Fo + 1], 1.0)
                    if Fo + 1 < 64:
                        nc.vector.memset(new_stage[:, jsl, Fo + 1:], 0.0)

                half = NB // 2 + 1
                off = 0
                for j in range(NB):
                    d = int(D[j])
                    g_t = gath_pool.tile([128, 1, d * 128], F16, tag="gath")
                    nc.gpsimd.dma_gather(
                        g_t[:], tab_sb[:].rearrange("p j f -> p (j f)"),
                        idx_sb[:, off:off + d * 8],
                        d * 128, d * 128, 128, single_packet=False,
                        transpose=True, sbuf_tokens_per_rank=128,
                        sbuf_free_dim_per_rank=256)
                    off += d * 8
                    # feat-major max over the d stride-128 token slots
                    m16 = m_pool.tile([128, 128], F16, tag="m16")
                    nc.vector.reduce_max(
                        m16[:],
                        g_t[:, 0, :].rearrange("q (d p) -> q p d", d=d),
                        axis=mybir.AxisListType.X)
                    # transpose back to node-major and widen to f32
                    pmt = psum_pool.tile([128, 128], F16, tag="pmt", bufs=2)
                    nc.tensor.matmul(pmt[:], m16[:], ident16[:],
                                     is_transpose=True)
                    nc.vector.tensor_copy(m_st[:, j, :Fo], pmt[:, :Fo])
                    if not last and j == half - 1:
                        pointwise(slice(0, half))

                if last:
                    msl = m_st[:, :NB, :Fo]
                    nc.vector.tensor_max(msl, msl, y_st[:, :NB, :Fo])
                    nc.vector.tensor_sub(msl, msl, y_st[:, :NB, :Fo])
                    fin = fin_pool.tile([128, NBV], F32)
                    nc.vector.tensor_add(m_st[:, :NB, 0:1], msl,
                                         z_st[:, :NB, :Fo])
                    nc.vector.tensor_scalar_max(fin[:], m_st[:, :NB, 0], 0.0)
                    nc.sync.dma_start(out_dram[:], fin[:])
                    break
                pointwise(slice(half, NB))
                if NB < NBG:
                    nc.vector.memset(new_stage[:, NB:, :], 0.0)
                stage = new_stage

                if l == _NLAYERS - 1 and dbg_dram is not None:
                    nc.sync.dma_start(dbg_dram[:, :NB, :], stage[:, :NB, :])
                    break

                # --- transition after b8: reshard g-space out -> v-space
                if l == 7:
                    ag2 = agin_pool.tile([NBG * 128, 64], F32, tag="agin")
                    nc.sync.dma_start(
                        ag2[:].rearrange("(j p) f -> p j f", p=128),
                        stage[:, :, :])
                    ttable = table_pool.tile([NC * NBG * 128, 64], F32,
                                             addr_space=_ADDR_SPACE,
                                             tag="ttable", name="ttable")
                    nc.gpsimd.collective_compute(
                        "AllGather", mybir.AluOpType.bypass,
                        replica_groups=[list(range(NC))],
                        ins=[ag2[:, :]], outs=[ttable[:]])
                    g_t = gath_pool.tile([128, NBV, 64], F32, tag="gath")
                    nc.gpsimd.dma_gather(
                        g_t[:], ttable[:], idxt_sb[:, :NBV * 8],
                        NBV * 128, NBV * 128, 64, single_packet=False)
                    stage_v = stage_pool.tile([128, NBG, 64], F32, tag="stage")
                    nc.vector.tensor_mul(stage_v[:, :NBV, :], g_t[:],
                                         mask_sb['u1'][:])
                    stage = stage_v

    nc.compile()
    return nc




def _bench_pjrt(nc, in_maps, n_iter=10):
    """Repeat execution with device-resident inputs; report per-run wall times.

    Mirrors bass2jax.run_bass_via_pjrt's multi-core path but keeps inputs on
    device so repeated calls measure launch + execute (not input shipping)."""
    import time as _time

    import jax
    from jax.sharding import Mesh, PartitionSpec
    from jax.experimental.shard_map import shard_map

    import concourse.mybir as _mybir
    from concourse import bass2jax as b2j

    b2j.install_neuronx_cc_hook()
    partition_name = nc.partition_id_tensor.name if nc.partition_id_tensor else None
    in_names, out_names, out_avals, zero_outs = [], [], [], []
    for alloc in nc.m.functions[0].allocations:
        if not isinstance(alloc, _mybir.MemoryLocationSet):
            continue
        name = alloc.memorylocations[0].name
        if alloc.kind == "ExternalInput":
            if name != partition_name:
                in_names.append(name)
        elif alloc.kind == "ExternalOutput":
            shape = tuple(alloc.tensor_shape)
            dtype = _mybir.dt.np(alloc.dtype)
            out_names.append(name)
            out_avals.append(jax.core.ShapedArray(shape, dtype))
            zero_outs.append(np.zeros(shape, dtype))
    n_params = len(in_names)
    n_outs = len(out_avals)
    all_names = list(in_names) + out_names + ([partition_name] if partition_name else [])

    def _body(*args):
        operands = list(args)
        if partition_name is not None:
            operands.append(b2j.partition_id_tensor())
        return tuple(b2j._bass_exec_p.bind(
            *operands, out_avals=tuple(out_avals), in_names=tuple(all_names),
            out_names=tuple(out_names), lowering_input_output_aliases=(),
            sim_require_finite=True, sim_require_nnan=True, nc=nc))

    devices = jax.devices()[:NC]
    mesh = Mesh(np.asarray(devices), ("core",))
    sharded = jax.jit(
        shard_map(_body, mesh=mesh,
                  in_specs=(PartitionSpec("core"),) * (n_params + n_outs),
                  out_specs=(PartitionSpec("core"),) * n_outs,
                  check_rep=False),
        donate_argnums=tuple(range(n_params, n_params + n_outs)),
        keep_unused=True)
    sharding = jax.sharding.NamedSharding(mesh, PartitionSpec("core"))
    dev_in = [jax.device_put(
        np.concatenate([np.asarray(m[k]) for m in in_maps], axis=0), sharding)
        for k in in_names]
    times = []
    for i in range(n_iter):
        zeros = [jax.device_put(
            np.zeros((NC * z.shape[0], *z.shape[1:]), z.dtype), sharding)
            for z in zero_outs]
        for z in zeros:
            z.block_until_ready()
        t0 = _time.perf_counter()
        outs = sharded(*dev_in, *zeros)
        for o in outs:
            o.block_until_ready()
        times.append(_time.perf_counter() - t0)
    return times


# ---------------------------------------------------------------- kernel

def kernel(**inputs):
    f32 = np.float32
    inp = {k: np.asarray(v) for k, v in inputs.items()}

    x0 = np.concatenate([
        np.concatenate([inp['inputs'][0].astype(f32),
                        np.zeros((N_VOX, 51), f32)], axis=0),
        inp['koor'].astype(f32)], axis=1)                  # [8600, 54]
    um = {k: ((inp[k] > 0.5).astype(f32) * 2.0) for k in ('u1', 'u2', 'u3', 'u4')}

    # Self-edges only contribute y_own to the max; padding slots gather the
    # node's own row and an explicit max(m, y) covers full-degree blocks, so
    # drop them from the token tables entirely.
    gm = inp['src'] != inp['dst']
    vm = inp['vsrc'] != inp['vdst']
    g_src, g_dst = inp['src'][gm], inp['dst'][gm]
    v_src, v_dst = inp['vsrc'][vm], inp['vdst'][vm]
    g_n2o, g_o2n, Dg = _grouping(g_dst, N_ALL, NBG)
    v_n2o, v_o2n, Dv = _grouping(v_dst, N_VOX, NBV)
    idx_g = _tables(g_src, g_dst, N_ALL, g_n2o, g_o2n, Dg, NBG)
    idx_v = _tables(v_src, v_dst, N_VOX, v_n2o, v_o2n, Dv, NBV)
    Tg, Tv = 128 * int(Dg.sum()), 128 * int(Dv.sum())

    # per-core staged x0 (node-major, ones col at 54)
    x0_st = []
    for r in range(NC):
        st = np.zeros((128, NBG, 64), f32)
        st[:, :, 54] = 1.0
        for j in range(NBG):
            nid0 = r * NBG * 128 + j * 128
            olds = g_n2o[nid0:nid0 + 128]
            sel = olds >= 0
            st[sel, j, :54] = x0[olds[sel]]
        x0_st.append(st)

    # transition gather tokens: v-new-id -> g-new-id (of global old id 600+v)
    idxt = []
    for r in range(NC):
        tok = np.zeros(NBV * 128, np.int16)
        for j in range(NBV):
            for_p = v_n2o[r * NBV * 128 + j * 128: r * NBV * 128 + j * 128 + 128]
            t = np.zeros(128, np.int64)
            sel = for_p >= 0
            t[sel] = g_o2n[N_PMT + for_p[sel]]
            tok[j * 128:(j + 1) * 128] = t.astype(np.int16)
        idxt.append(_pack16(tok))

    # per-core masks (node-major padded)
    def mk_mask(u, F, ones_col):
        per = []
        for r in range(NC):
            mt = np.zeros((128, NBV, 64), f32)
            if ones_col is not None:
                mt[:, :, ones_col] = 1.0
            for j in range(NBV):
                olds = v_n2o[r * NBV * 128 + j * 128: r * NBV * 128 + j * 128 + 128]
                sel = olds >= 0
                mt[sel, j, :F] = u[olds[sel], :F]
            per.append(mt)
        return per
    m_u1 = mk_mask(um['u1'], 54, 54)
    m_u2 = mk_mask(um['u2'], 54, None)
    m_u3 = mk_mask(um['u3'], 25, None)
    m_u4 = mk_mask(um['u4'], 25, None)

    # weights: [64, 46*64]; layer l: wt at col 2l*64, wp_aug at (2l+1)*64
    descs = _layer_descs()
    wts = ([(inp['bwt'][i], inp['bbt'][i], inp['bwp'][i], inp['bbp'][i]) for i in range(8)]
           + [(inp['awt'][i], inp['abt'][i], inp['awp'][i], inp['abp'][i]) for i in range(4)]
           + [(inp['a5wt'], inp['a5bt'], inp['a5wp'], inp['a5bp'])]
           + [(inp['cwt'][i], inp['cbt'][i], inp['cwp'][i], inp['cbp'][i]) for i in range(9)]
           + [(inp['fwt'], inp['fbt'], inp['fwp'], inp['fbp'])])
    W = np.zeros((64, 46 * 64), f32)
    for l, (wt, bt, wp, bp) in enumerate(wts):
        Fi, Fo = wt.shape
        W[:Fi, 2 * l * 64:2 * l * 64 + Fo] = wt
        W[:Fi, (2 * l + 1) * 64:(2 * l + 1) * 64 + Fo] = wp
        W[Fi, (2 * l + 1) * 64:(2 * l + 1) * 64 + Fo] = bt + bp

    nc = _build(Dg, Dv, Tg, Tv, descs)

    in_maps = []
    for r in range(NC):
        in_maps.append({
            "x0": x0_st[r],
            "w": W,
            "idxg": _pack16(idx_g[r]),
            "idxv": _pack16(idx_v[r]),
            "idxt": idxt[r],
            "u1": m_u1[r], "u2": m_u2[r], "u3": m_u3[r], "u4": m_u4[r],
        })
    res = run_bass_kernel_spmd(nc, in_maps, core_ids=list(range(NC)))
    kernel.last_results = res
    nbench = int(os.environ.get("KERNEL_BENCH", "0"))
    if nbench:
        times = _bench_pjrt(nc, in_maps, nbench)
        kernel.bench_times = times
        print("bench ms:", " ".join(f"{t*1e3:.2f}" for t in times))

    out = np.zeros(N_VOX, f32)
    for r in range(NC):
        o = res.results[r]["out"]                  # [128, NBV]
        for j in range(NBV):
            olds = v_n2o[r * NBV * 128 + j * 128: r * NBV * 128 + j * 128 + 128]
            sel = olds >= 0
            out[olds[sel]] = o[sel, j]
    return out


if __name__ == "__main__":
    d = np.load('/tmp/inputs.npz')
    inputs = {k: d[k] for k in d.files}
    expected = np.load('/tmp/expected.npy')
    got = kernel(**inputs)
    rel = np.linalg.norm(got - expected) / np.linalg.norm(expected)
    print("rel_l2:", rel)

